# revision 23
# baseline (speedup 1.0000x reference)
"""NonLocal2D (attention) block on 8 trn2 NeuronCores.

Sharding: core c -> batch n = c//2, query-half qh = c%2 (2048 of the 4096
spatial positions). Each core receives the full x[n] (so phi/g are computed
locally -- no collectives) plus its own query slice, and produces
out[n][:, qh*2048:(qh+1)*2048].

Per-core dataflow (layouts chosen so no transposes are ever needed):
  theta:    [CI=128, Q]  = wthT-chunks (lhsT) @ xt-chunks (rhs)     [PE]
  phi:      [CI=128, N]  = wphT-chunks (lhsT) @ xb-chunks (rhs)     [PE]
  g^T:      [s, CI] tiles = xb-chunks (lhsT) @ wgT-chunks (rhs)     [PE]
  scores^T: [s=128, q=1024] = phi-tile (lhsT) @ theta (rhs)         [PE -> PSUM f32]
  B = exp(SCALE*scores^T) -> bf16 SBUF; most half-tiles on ACT, a
      subset on DVE via the Schraudolph int16-bits trick (bf16 bits =
      trunc(x*128/ln2 + 127*128 - c))  (no max-sub: |scaled| < ~30)  [ACT+DVE]
  denom: 4 stride-4 fold chains over B tiles (split GPSIMD/DVE via a
      static LUT; the first add of each chain combines two B tiles so
      no copies are needed), combined to one F, then
      d = ones (lhsT) @ F per q-chunk                               [DVE/GPSIMD/PE]
  y^T += gT-tile (lhsT) @ B   (PSUM accumulate over 32 s-tiles)     [PE]
  y_norm^T = y^T * (1/d) -> bf16                                    [DVE]
  out-proj + residual in one PSUM group:
      rp = ident (lhsT) @ xt-chunk  (start)  -- the +x residual
      rp += woT-chunk (lhsT) @ y_norm^T (stop); DMA out from PSUM   [PE]

The residual uses the bf16 xt copy (no separate f32 x load): the extra
~2^-9 relative rounding on x costs ~1.7e-3 rel err, far under the 2e-2
gate, and saves 4MB/core of input DMA.
"""

import numpy as np
import ml_dtypes

import concourse.bass as bass
import concourse.mybir as mybir
import concourse.tile as tile
from concourse import bacc
from concourse.bass import ts
from concourse.bass_utils import run_bass_kernel_spmd

BF16 = mybir.dt.bfloat16
F32 = mybir.dt.float32
I16 = mybir.dt.int16
AF = mybir.ActivationFunctionType
ALU = mybir.AluOpType

C = 256          # in channels
CI = 128         # inter channels
NB = 4           # batch
N = 4096         # H*W
Q = 2048         # queries per core
NCORES = 8
SCALE = float(128 ** 0.5)   # reference divides by d**-0.5

# Schraudolph exp in bf16-bit space: bits = trunc(x*SCALE*128/ln2 + b)
SCH_A = SCALE * 128.0 / float(np.log(2.0))
SCH_B = 127.0 * 128.0 - 5.0

N_BSLOT = 24     # B-tile ring (WAR distance 24 >> pipeline depth)

# fold engine LUT: which tiles' fold-adds run on GPSIMD (rest on DVE).
# Exactly chain 0 (j = i%4 == 0): a whole chain per engine, so no
# cross-engine handoff ever head-blocks the DVE queue.
FOLD_POOL = {4, 8, 12, 16, 20, 24, 28}

_CACHE: dict = {}


def _dve_exp_half(i, h):
    # which exp half-tiles run on DVE (Schraudolph) instead of ACT.
    # Tiles >= 28 stay on ACT: at that point DVE is running the fold
    # combines that gate the tail, and the last exp must not queue there.
    return h == 0 and i % 3 == 1 and i < 28


def _build(flags):
    bth_nz, bph_nz, bg_nz, bo_nz = flags
    nc = bacc.Bacc("TRN2", target_bir_lowering=False, debug=False)

    d = {}
    d["xb"] = nc.dram_tensor("xb", [2, 128, N], BF16, kind="ExternalInput").ap()
    d["xt"] = nc.dram_tensor("xt", [2, 128, Q], BF16, kind="ExternalInput").ap()
    d["wthT"] = nc.dram_tensor("wthT", [2, 128, CI], BF16, kind="ExternalInput").ap()
    d["wphT"] = nc.dram_tensor("wphT", [2, 128, CI], BF16, kind="ExternalInput").ap()
    d["wgT"] = nc.dram_tensor("wgT", [2, 128, CI], BF16, kind="ExternalInput").ap()
    d["woT"] = nc.dram_tensor("woT", [128, C], BF16, kind="ExternalInput").ap()
    d["ident"] = nc.dram_tensor("ident", [128, 128], BF16, kind="ExternalInput").ap()
    d["bth"] = nc.dram_tensor("bth", [128, 1], F32, kind="ExternalInput").ap() if bth_nz else None
    d["bph"] = nc.dram_tensor("bph", [128, 1], F32, kind="ExternalInput").ap() if bph_nz else None
    d["bg"] = nc.dram_tensor("bg", [1, CI], F32, kind="ExternalInput").ap() if bg_nz else None
    d["bo"] = nc.dram_tensor("bo", [2, 128, 1], F32, kind="ExternalInput").ap() if bo_nz else None
    d["out"] = nc.dram_tensor("out", [2, 128, Q], F32, kind="ExternalOutput").ap()

    with tile.TileContext(nc) as tc:
        _bass_body(tc, d)
    nc.compile()
    return nc


def _kc_pair_ap(dram_ap, cols, col0, count):
    """3D dram AP reading [2,128,cols] as [p=128, kc=2, count] at col0."""
    return bass.AP(
        tensor=dram_ap.tensor,
        offset=col0,
        ap=[[cols, 128], [128 * cols, 2], [1, count]],
    )


def _bass_body(tc, d):
    nc = tc.nc

    with (
        tc.tile_pool(name="const", bufs=1) as const,
        tc.tile_pool(name="acts", bufs=1) as acts,
        tc.tile_pool(name="outs", bufs=2) as outp,
    ):
        # ---- constants / weights ----
        # memsets on DVE (idle at t=0) so the PE warm-up never waits on the
        # gpsimd program load; the exp-table warm reads scratch itself
        # (garbage in, table warmed) so it carries no cross-engine dep
        ones_sb = const.tile([128, 128], BF16, tag="ones")
        nc.vector.memset(ones_sb[:], 1.0)
        wup_rhs = const.tile([128, 512], BF16, tag="wup_rhs")
        nc.vector.memset(wup_rhs[:], 0.0)
        scratch = const.tile([128, 1], BF16, tag="scratch")
        nc.vector.memset(scratch[:], 1.0)

        wth_sb = const.tile([128, 2, CI], BF16, tag="wth")
        wph_sb = const.tile([128, 2, CI], BF16, tag="wph")
        wg_sb = const.tile([128, 2, CI], BF16, tag="wg")
        wo_sb = const.tile([128, C], BF16, tag="wo")
        id_sb = const.tile([128, 128], BF16, tag="ident")
        bth_sb = bph_sb = bg_sb = bo_sb = None
        if d["bth"] is not None:
            bth_sb = const.tile([128, 1], F32, tag="bth")
        if d["bph"] is not None:
            bph_sb = const.tile([128, 1], F32, tag="bph")
        if d["bg"] is not None:
            bg_sb = const.tile([1, CI], F32, tag="bg")
        if d["bo"] is not None:
            bo_sb = const.tile([128, 2, 1], F32, tag="bo")

        th_sb = acts.tile([128, Q], BF16, tag="th")
        ph_sb = acts.tile([128, N], BF16, tag="ph")
        gT_sb = acts.tile([128, 32 * CI], BF16, tag="gT")  # tile i at cols [128i, 128i+128)

        # ---- input fill: one tile per DMA transfer so every consumer's
        # RAW dep is exactly one transfer (no waiting on sibling chunks),
        # ordered by first use across three queues ----
        xin_cm = tc.tile_pool(name="xin", bufs=1)
        xin = xin_cm.__enter__()
        xt_t = [
            xin.tile([128, 2, 1024], BF16, tag=f"xt{hh}", name=f"xt{hh}")
            for hh in range(2)
        ]
        xbf_sb = xin.tile([128, 2, 2048], BF16, tag="xbf")
        xbb_sb = xin.tile([128, 2, 2048], BF16, tag="xbb")

        # sync: xt halves (theta)
        for hh in range(2):
            nc.sync.dma_start(
                out=xt_t[hh][:],
                in_=_kc_pair_ap(d["xt"], Q, hh * 1024, 1024))
        nc.sync.dma_start(
            out=xbb_sb[:], in_=_kc_pair_ap(d["xb"], N, 2048, 2048))

        # scalar: weights (wth first for theta), each as one transfer
        nc.scalar.dma_start(out=wth_sb[:], in_=_kc_pair_ap(d["wthT"], CI, 0, CI))
        nc.scalar.dma_start(out=wph_sb[:], in_=_kc_pair_ap(d["wphT"], CI, 0, CI))
        nc.scalar.dma_start(out=wg_sb[:], in_=_kc_pair_ap(d["wgT"], CI, 0, CI))
        nc.scalar.dma_start(out=wo_sb[:], in_=d["woT"][:])
        nc.scalar.dma_start(out=id_sb[:], in_=d["ident"][:])
        if bth_sb is not None:
            nc.scalar.dma_start(out=bth_sb[:], in_=d["bth"][:])
        if bph_sb is not None:
            nc.scalar.dma_start(out=bph_sb[:], in_=d["bph"][:])
        if bg_sb is not None:
            nc.scalar.dma_start(out=bg_sb[:], in_=d["bg"][:])
        if bo_sb is not None:
            for oc in range(2):
                nc.scalar.dma_start(out=bo_sb[:, oc, :], in_=d["bo"][oc])

        # gpsimd: front half of xb (phi hh0/1)
        nc.gpsimd.dma_start(
            out=xbf_sb[:], in_=_kc_pair_ap(d["xb"], N, 0, 2048))

        def xb_at(st128):
            # (tile, local col0) for xb column st128*128
            col = st128 * 128
            return (xbf_sb, col) if col < 2048 else (xbb_sb, col - 2048)

        # warm the exp table set early (after the weight DMA issues, so
        # those aren't queued behind the 1.3us table load)
        nc.scalar.activation(scratch[:], scratch[:], AF.Exp, scale=1.0)

        def cast_out(dst_ap, src_psum, bias_part, bias_row):
            # PSUM f32 -> SBUF bf16, optionally + bias
            if bias_part is not None:
                nc.vector.tensor_scalar_add(dst_ap, src_psum, bias_part[:])
            elif bias_row is not None:
                bcast = bass.AP(
                    tensor=bias_row.tensor,
                    offset=bias_row.offset,
                    ap=[[0, 128], [0, 4], [1, CI]],
                )
                nc.vector.tensor_tensor(dst_ap, src_psum, bcast, ALU.add)
            else:
                nc.vector.tensor_copy(dst_ap, src_psum)

        # ---- theta + first quarter of phi (enough for 8 s-tiles) ----
        with (
            tc.tile_pool(name="pj", bufs=2, space="PSUM") as pj,
            tc.tile_pool(name="wup", bufs=1, space="PSUM") as wup,
        ):
            # PE warm-up during the DMA fill: dummy matmuls flip the HAM
            # clock gate toward 8/8 before the first real matmul issues;
            # 16 of them (~5-6us) bridge until the xt/wth DMAs land
            wps = wup.tile([128, 512], F32, tag="wps")
            for _ in range(16):
                nc.tensor.matmul(
                    wps[:, 0:256], ones_sb[:], wup_rhs[:, 0:256],
                    start=True, stop=True)

            def theta_round(hh):
                tp = pj.tile([128, 1024], F32, tag="pj", name=f"tp{hh}")
                for qc in range(2):
                    for kc in range(2):
                        nc.tensor.matmul(
                            tp[:, ts(qc, 512)],
                            wth_sb[:, kc, :],
                            xt_t[hh][:, kc, ts(qc, 512)],
                            start=(kc == 0),
                            stop=(kc == 1),
                        )
                cast_out(th_sb[:, ts(hh, 1024)], tp[:], bth_sb, None)

            # theta-hh0 -> phi-hh0 -> theta-hh1, so the DVE cast chain for
            # the first exp never waits on a later DMA chunk
            theta_round(0)
            pp = pj.tile([128, 1024], F32, tag="pj")
            for qc in range(2):
                for kc in range(2):
                    nc.tensor.matmul(
                        pp[:, ts(qc, 512)],
                        wph_sb[:, kc, :],
                        xbf_sb[:, kc, ts(qc, 512)],
                        start=(kc == 0),
                        stop=(kc == 1),
                    )
            cast_out(ph_sb[:, 0:1024], pp[:], bph_sb, None)
            theta_round(1)

        # ---- attention, software-pipelined against the remaining
        # projections: exp for s-tile i+8 is emitted behind the y-matmuls
        # of tile i, and the first 8 score/exp pairs precede the phi tail and
        # the whole g^T phase. phi-tail and g^T borrow the yps PSUM banks
        # (the y accumulation's start=True clears them afterwards).
        fF = [
            acts.tile([128, Q], BF16, tag=f"F{j}", name=f"F{j}")
            for j in range(4)
        ]
        ypsp_cm = tc.tile_pool(name="yps", bufs=1, space="PSUM")
        ypsp = ypsp_cm.__enter__()
        yps = ypsp.tile([128, Q], F32, tag="yps")
        scp_cm = tc.tile_pool(name="scp", bufs=2, space="PSUM")
        scp = scp_cm.__enter__()
        bp_cm = tc.tile_pool(name="bp", bufs=1)
        bp = bp_cm.__enter__()
        Bt = {}

        def sc_exp(i):
            B = bp.tile([128, Q], BF16, tag=f"B{i % N_BSLOT}", name=f"B{i}")
            Bt[i] = B
            for h in range(2):
                sc = scp.tile([128, 1024], F32, tag="sc")
                for qc in range(2):
                    nc.tensor.matmul(
                        sc[:, ts(qc, 512)],
                        ph_sb[:, ts(i, 128)],
                        th_sb[:, ts(h * 2 + qc, 512)],
                        start=True,
                        stop=True,
                    )
                if _dve_exp_half(i, h):
                    # Schraudolph exp on DVE: bf16 bits of exp(SCALE*sc)
                    nc.vector.tensor_scalar(
                        B[:, ts(h, 1024)].bitcast(I16), sc[:],
                        SCH_A, SCH_B, ALU.mult, ALU.add)
                else:
                    nc.scalar.activation(
                        B[:, ts(h, 1024)], sc[:], AF.Exp, scale=SCALE)
            # stride-4 fold chains, engine per FOLD_POOL LUT. The first add
            # of a chain sums B[j] and B[j+4] directly -- no seed copies.
            # Tile 31 is NOT folded into chain 3: it becomes the single
            # final add after the chain combines, so the tail critical path
            # past the last exp is one add. DVE folds for tiles 4..7 are
            # emitted later (after the g casts) so the g casts -- which gate
            # the whole y main loop -- aren't queued behind them.
            j = i % 4
            if i < 4 or i == 31:
                pass
            elif i in FOLD_POOL:
                if i == 4:
                    nc.gpsimd.tensor_tensor(fF[j][:], Bt[0][:], Bt[4][:], ALU.add)
                else:
                    nc.gpsimd.tensor_tensor(fF[j][:], fF[j][:], Bt[i][:], ALU.add)
            elif i >= 8:
                nc.vector.tensor_tensor(fF[j][:], fF[j][:], Bt[i][:], ALU.add)

        for i in range(8):
            sc_exp(i)

        # phi tail (tiles 8..31) into borrowed yps banks
        for hh in range(1, 4):
            pp = yps[:, ts(hh % 2, 1024)]
            xbt = xbf_sb if hh < 2 else xbb_sb
            lc = (hh % 2) * 1024
            for qc in range(2):
                for kc in range(2):
                    nc.tensor.matmul(
                        pp[:, ts(qc, 512)],
                        wph_sb[:, kc, :],
                        xbt[:, kc, lc + qc * 512:lc + qc * 512 + 512],
                        start=(kc == 0),
                        stop=(kc == 1),
                    )
            cast_out(ph_sb[:, ts(hh, 1024)], pp[:], bph_sb, None)

        # g^T projection, also into borrowed yps banks
        for b in range(8):
            gp = yps[:, 512 * (b % 4):512 * (b % 4) + 512]
            for sj in range(4):
                st = b * 4 + sj
                xbt, lc = xb_at(st)
                for kc in range(2):
                    nc.tensor.matmul(
                        gp[:, ts(sj, 128)],
                        xbt[:, kc, lc:lc + 128],
                        wg_sb[:, kc, :],
                        start=(kc == 0),
                        stop=(kc == 1),
                    )
            cast_out(gT_sb[:, ts(b, 512)], gp[:], None, bg_sb)

        # deferred DVE first-adds for chains 1..3 (tiles 5,6,7), after the
        # g casts so those never wait behind exp-gated folds
        for i in (5, 6, 7):
            nc.vector.tensor_tensor(fF[i % 4][:], Bt[i % 4][:], Bt[i][:], ALU.add)

        # ---- main loop: y(i) first, then scores/exp for i+8, so the PE
        # queue head never blocks on an exp that is still in flight ----
        for i in range(32):
            B = Bt[i]
            for h in range(2):
                for qc in range(2):
                    nc.tensor.matmul(
                        yps[:, ts(h * 2 + qc, 512)],
                        gT_sb[:, ts(i, 128)],
                        B[:, ts(h * 2 + qc, 512)],
                        start=(i == 0),
                        stop=(i == 31),
                    )
            if i < 24:
                sc_exp(i + 8)
            # pre-emit the chain combines so only one add remains after
            # the last exp: c1 = F1+F0 right after fold(29) (GPSIMD chain
            # 0 ended at 28), c2 = F2+F3 and c3 after fold(30)
            if i == 21:
                nc.vector.tensor_tensor(fF[1][:], fF[1][:], fF[0][:], ALU.add)
            elif i == 22:
                nc.vector.tensor_tensor(fF[2][:], fF[2][:], fF[3][:], ALU.add)
                nc.vector.tensor_tensor(fF[1][:], fF[1][:], fF[2][:], ALU.add)
        # the single final add of tile 31's B right after its exp lands
        nc.vector.tensor_tensor(fF[1][:], fF[1][:], Bt[31][:], ALU.add)
        bp_cm.__exit__(None, None, None)
        scp_cm.__exit__(None, None, None)

        # ---- tail, pipelined per 512-wide q-chunk:
        # d (1 MM over F) -> 1/d (approx) -> y*1/d -> {residual + out-proj
        # as one PSUM accumulation: ident@xt (start) + woT@ynt (stop)} ->
        # DMA out straight from PSUM.
        with (
            tc.tile_pool(name="dps", bufs=2, space="PSUM") as dpsp,
            tc.tile_pool(name="rps", bufs=2, space="PSUM") as rps,
        ):
            for qc in range(4):
                dp = dpsp.tile([128, 512], F32, tag="dp")
                nc.tensor.matmul(
                    dp[:], ones_sb[:], fF[1][:, ts(qc, 512)],
                    start=True, stop=True)
                rcp = outp.tile([128, 512], F32, tag="rcp")
                nc.vector.reciprocal_approx_fast(rcp[:], dp[:])
                ynt = outp.tile([128, 512], BF16, tag="ynt")
                nc.vector.tensor_tensor(
                    ynt[:], yps[:, ts(qc, 512)], rcp[:], ALU.mult)
                for oc in range(2):
                    rp = rps.tile([128, 512], F32, tag="rp")
                    xres = xt_t[qc // 2][:, oc, ts(qc % 2, 512)]
                    nc.tensor.matmul(
                        rp[:], id_sb[:], xres,
                        start=True, stop=False)
                    nc.tensor.matmul(
                        rp[:],
                        wo_sb[:, ts(oc, 128)],
                        ynt[:],
                        start=False,
                        stop=True,
                    )
                    # PSUM -> SBUF split across ACT (oc0) and DVE (oc1) so
                    # the copies run concurrently (DMA cannot read PSUM);
                    # out DMAs split across the sync and scalar queues
                    ot = outp.tile([128, 512], F32, tag=f"ot{oc}")
                    if bo_sb is not None:
                        nc.scalar.activation(
                            ot[:], rp[:], AF.Identity, bias=bo_sb[:, oc, :])
                    elif oc == 0:
                        nc.scalar.copy(ot[:], rp[:])
                    else:
                        nc.vector.tensor_copy(ot[:], rp[:])
                    [nc.sync, nc.scalar][oc].dma_start(
                        out=d["out"][oc][:, ts(qc, 512)], in_=ot[:])
        ypsp_cm.__exit__(None, None, None)
        xin_cm.__exit__(None, None, None)


def _prep_in_maps(inputs):
    bf = ml_dtypes.bfloat16
    x = np.ascontiguousarray(np.asarray(inputs["x"], dtype=np.float32))
    w_g = np.asarray(inputs["w_g"], np.float32)
    b_g = np.asarray(inputs["b_g"], np.float32)
    w_theta = np.asarray(inputs["w_theta"], np.float32)
    b_theta = np.asarray(inputs["b_theta"], np.float32)
    w_phi = np.asarray(inputs["w_phi"], np.float32)
    b_phi = np.asarray(inputs["b_phi"], np.float32)
    w_out = np.asarray(inputs["w_out"], np.float32)
    b_out = np.asarray(inputs["b_out"], np.float32)

    flags = (
        bool(np.any(b_theta)), bool(np.any(b_phi)),
        bool(np.any(b_g)), bool(np.any(b_out)),
    )
    wthT = np.ascontiguousarray(w_theta.T).astype(bf).reshape(2, 128, CI)
    wphT = np.ascontiguousarray(w_phi.T).astype(bf).reshape(2, 128, CI)
    wgT = np.ascontiguousarray(w_g.T).astype(bf).reshape(2, 128, CI)
    woT = np.ascontiguousarray(w_out.T).astype(bf)          # [CI, C]
    ident = np.eye(128, dtype=bf)

    in_maps = []
    for c in range(NCORES):
        n, qh = c // 2, c % 2
        xr = x[n].reshape(C, N)
        xbc = xr.astype(bf)
        m = {
            "xb": np.ascontiguousarray(xbc.reshape(2, 128, N)),
            "xt": np.ascontiguousarray(
                xbc[:, qh * Q:(qh + 1) * Q].reshape(2, 128, Q)),
            "wthT": wthT, "wphT": wphT, "wgT": wgT, "woT": woT,
            "ident": ident,
        }
        if flags[0]:
            m["bth"] = np.ascontiguousarray(b_theta.reshape(128, 1))
        if flags[1]:
            m["bph"] = np.ascontiguousarray(b_phi.reshape(128, 1))
        if flags[2]:
            m["bg"] = np.ascontiguousarray(b_g.reshape(1, CI))
        if flags[3]:
            m["bo"] = np.ascontiguousarray(b_out.reshape(2, 128, 1))
        in_maps.append(m)
    return flags, in_maps


def _get_nc(flags):
    if flags not in _CACHE:
        _CACHE[flags] = _build(flags)
    return _CACHE[flags]


def kernel(**inputs):
    flags, in_maps = _prep_in_maps(inputs)
    nc = _get_nc(flags)
    res = run_bass_kernel_spmd(nc, in_maps, list(range(NCORES)))
    out = np.empty((NB, C, N), np.float32)
    for c in range(NCORES):
        n, qh = c // 2, c % 2
        out[n][:, qh * Q:(qh + 1) * Q] = res.results[c]["out"].reshape(C, Q)
    return out.reshape(NB, C, 64, 64)


if __name__ == "__main__":
    x = np.random.randn(NB, C, 64, 64).astype(np.float32) * 0.1
    rng = np.random.default_rng(0)
    ins = {
        "x": x,
        "w_g": rng.normal(size=(CI, C)).astype(np.float32) * 0.01,
        "b_g": np.zeros(CI, np.float32),
        "w_theta": rng.normal(size=(CI, C)).astype(np.float32) * 0.01,
        "b_theta": np.zeros(CI, np.float32),
        "w_phi": rng.normal(size=(CI, C)).astype(np.float32) * 0.01,
        "b_phi": np.zeros(CI, np.float32),
        "w_out": rng.normal(size=(C, CI)).astype(np.float32) * 0.01,
        "b_out": np.zeros(C, np.float32),
    }
    o = kernel(**ins)
    print("ok", o.shape, o.dtype)


# revision 28
# speedup vs baseline: 1.1751x; 1.1751x over previous
"""NonLocal2D (attention) block on 8 trn2 NeuronCores.

Sharding: core c -> batch n = c//2, query-half qh = c%2 (2048 of the 4096
spatial positions). Each core receives the full x[n] (so phi/g are computed
locally -- no collectives) plus its own query slice, and produces
out[n][:, qh*2048:(qh+1)*2048].

Per-core dataflow (layouts chosen so no transposes are ever needed):
  theta:    [CI=128, Q]  = wthT-chunks (lhsT) @ xt-chunks (rhs)     [PE]
  phi:      [CI=128, N]  = wphT-chunks (lhsT) @ xb-chunks (rhs)     [PE]
  g^T:      [s, CI] tiles = xb-chunks (lhsT) @ wgT-chunks (rhs)     [PE]
  scores^T: [s=128, q=1024] = phi-tile (lhsT) @ theta (rhs)         [PE -> PSUM f32]
  B = exp(SCALE*scores^T) -> bf16 SBUF; most half-tiles on ACT, a
      subset on DVE via the Schraudolph int16-bits trick (bf16 bits =
      trunc(x*128/ln2 + 127*128 - c))  (no max-sub: |scaled| < ~30)  [ACT+DVE]
  denom: 4 stride-4 fold chains over B tiles (split GPSIMD/DVE via a
      static LUT; the first add of each chain combines two B tiles so
      no copies are needed), combined to one F, then
      d = ones (lhsT) @ F per q-chunk                               [DVE/GPSIMD/PE]
  y^T += gT-tile (lhsT) @ B   (PSUM accumulate over 32 s-tiles)     [PE]
  y_norm^T = y^T * (1/d) -> bf16                                    [DVE]
  out-proj + residual in one PSUM group:
      rp = ident (lhsT) @ xt-chunk  (start)  -- the +x residual
      rp += woT-chunk (lhsT) @ y_norm^T (stop); DMA out from PSUM   [PE]

The residual uses the bf16 xt copy (no separate f32 x load): the extra
~2^-9 relative rounding on x costs ~1.7e-3 rel err, far under the 2e-2
gate, and saves 4MB/core of input DMA.
"""

import numpy as np
import ml_dtypes

import concourse.bass as bass
import concourse.mybir as mybir
import concourse.tile as tile
from concourse import bacc
from concourse.bass import ts
from concourse.bass_utils import run_bass_kernel_spmd

BF16 = mybir.dt.bfloat16
F32 = mybir.dt.float32
I16 = mybir.dt.int16
AF = mybir.ActivationFunctionType
ALU = mybir.AluOpType

C = 256          # in channels
CI = 128         # inter channels
NB = 4           # batch
N = 4096         # H*W
Q = 2048         # queries per core
NCORES = 8
SCALE = float(128 ** 0.5)   # reference divides by d**-0.5

# Schraudolph exp in bf16-bit space: bits = trunc(x*SCALE*128/ln2 + b)
SCH_A = SCALE * 128.0 / float(np.log(2.0))
SCH_B = 127.0 * 128.0 - 5.0

N_BSLOT = 24     # B-tile ring (WAR distance 24 >> pipeline depth)

# fold engine LUT: which tiles' fold-adds run on GPSIMD (rest on DVE).
# Exactly chain 0 (j = i%4 == 0): a whole chain per engine, so no
# cross-engine handoff ever head-blocks the DVE queue.
FOLD_POOL = {4, 8, 12, 16, 20, 24, 28}

_CACHE: dict = {}


def _dve_exp_half(i, h):
    # which exp half-tiles run on DVE (Schraudolph) instead of ACT.
    # Tiles >= 28 stay on ACT: at that point DVE is running the fold
    # combines that gate the tail, and the last exp must not queue there.
    return h == 0 and i % 3 == 1 and i < 28


def _build(flags):
    bth_nz, bph_nz, bg_nz, bo_nz = flags
    nc = bacc.Bacc("TRN2", target_bir_lowering=False, debug=False)

    d = {}
    d["xb"] = nc.dram_tensor("xb", [2, 128, N], BF16, kind="ExternalInput").ap()
    d["xt"] = nc.dram_tensor("xt", [2, 128, Q], BF16, kind="ExternalInput").ap()
    d["wthT"] = nc.dram_tensor("wthT", [2, 128, CI], BF16, kind="ExternalInput").ap()
    d["wphT"] = nc.dram_tensor("wphT", [2, 128, CI], BF16, kind="ExternalInput").ap()
    d["wgT"] = nc.dram_tensor("wgT", [2, 128, CI], BF16, kind="ExternalInput").ap()
    d["woT"] = nc.dram_tensor("woT", [128, C], BF16, kind="ExternalInput").ap()
    d["ident"] = nc.dram_tensor("ident", [128, 128], BF16, kind="ExternalInput").ap()
    d["bth"] = nc.dram_tensor("bth", [128, 1], F32, kind="ExternalInput").ap() if bth_nz else None
    d["bph"] = nc.dram_tensor("bph", [128, 1], F32, kind="ExternalInput").ap() if bph_nz else None
    d["bg"] = nc.dram_tensor("bg", [1, CI], F32, kind="ExternalInput").ap() if bg_nz else None
    d["bo"] = nc.dram_tensor("bo", [2, 128, 1], F32, kind="ExternalInput").ap() if bo_nz else None
    d["out"] = nc.dram_tensor("out", [2, 128, Q], F32, kind="ExternalOutput").ap()

    with tile.TileContext(nc) as tc:
        _bass_body(tc, d)
    nc.compile()
    return nc


def _kc_pair_ap(dram_ap, cols, col0, count):
    """3D dram AP reading [2,128,cols] as [p=128, kc=2, count] at col0."""
    return bass.AP(
        tensor=dram_ap.tensor,
        offset=col0,
        ap=[[cols, 128], [128 * cols, 2], [1, count]],
    )


def _bass_body(tc, d):
    nc = tc.nc

    with (
        tc.tile_pool(name="const", bufs=1) as const,
        tc.tile_pool(name="acts", bufs=1) as acts,
        tc.tile_pool(name="outs", bufs=2) as outp,
    ):
        # ---- constants / weights ----
        # memsets on DVE (idle at t=0) so the PE warm-up never waits on the
        # gpsimd program load; the exp-table warm reads scratch itself
        # (garbage in, table warmed) so it carries no cross-engine dep
        ones_sb = const.tile([128, 128], BF16, tag="ones")
        nc.vector.memset(ones_sb[:], 1.0)
        wup_rhs = const.tile([128, 512], BF16, tag="wup_rhs")
        nc.vector.memset(wup_rhs[:], 0.0)
        scratch = const.tile([128, 1], BF16, tag="scratch")
        nc.vector.memset(scratch[:], 1.0)

        wth_sb = const.tile([128, 2, CI], BF16, tag="wth")
        wph_sb = const.tile([128, 2, CI], BF16, tag="wph")
        wg_sb = const.tile([128, 2, CI], BF16, tag="wg")
        wo_sb = const.tile([128, C], BF16, tag="wo")
        id_sb = const.tile([128, 128], BF16, tag="ident")
        bth_sb = bph_sb = bg_sb = bo_sb = None
        if d["bth"] is not None:
            bth_sb = const.tile([128, 1], F32, tag="bth")
        if d["bph"] is not None:
            bph_sb = const.tile([128, 1], F32, tag="bph")
        if d["bg"] is not None:
            bg_sb = const.tile([1, CI], F32, tag="bg")
        if d["bo"] is not None:
            bo_sb = const.tile([128, 2, 1], F32, tag="bo")

        th_sb = acts.tile([128, Q], BF16, tag="th")
        ph_sb = acts.tile([128, N], BF16, tag="ph")
        gT_sb = acts.tile([128, 32 * CI], BF16, tag="gT")  # tile i at cols [128i, 128i+128)

        # ---- input fill: one tile per DMA transfer so every consumer's
        # RAW dep is exactly one transfer (no waiting on sibling chunks),
        # ordered by first use across three queues ----
        xin_cm = tc.tile_pool(name="xin", bufs=1)
        xin = xin_cm.__enter__()
        xt_t = [
            xin.tile([128, 2, 1024], BF16, tag=f"xt{hh}", name=f"xt{hh}")
            for hh in range(2)
        ]
        xbf_sb = xin.tile([128, 2, 2048], BF16, tag="xbf")
        xbb_sb = xin.tile([128, 2, 2048], BF16, tag="xbb")

        # Only the transfers needed FIRST are issued here. DMA waits appear
        # to coalesce to "every transfer issued so far on that queue", so
        # later-needed tensors (wg, wo, ident, xb back half, biases) are
        # issued lazily at their point of need to keep theta's wait minimal.
        # sync: xt halves (theta); scalar: wth; gpsimd: front half of xb
        for hh in range(2):
            nc.sync.dma_start(
                out=xt_t[hh][:],
                in_=_kc_pair_ap(d["xt"], Q, hh * 1024, 1024))
        nc.scalar.dma_start(out=wth_sb[:], in_=_kc_pair_ap(d["wthT"], CI, 0, CI))
        if bth_sb is not None:
            nc.scalar.dma_start(out=bth_sb[:], in_=d["bth"][:])
        if bph_sb is not None:
            nc.scalar.dma_start(out=bph_sb[:], in_=d["bph"][:])
        nc.gpsimd.dma_start(
            out=xbf_sb[:], in_=_kc_pair_ap(d["xb"], N, 0, 2048))

        def xb_at(st128):
            # (tile, local col0) for xb column st128*128
            col = st128 * 128
            return (xbf_sb, col) if col < 2048 else (xbb_sb, col - 2048)

        # warm the exp table set early so the first real exp isn't +2.7us
        nc.scalar.activation(scratch[:], scratch[:], AF.Exp, scale=1.0)


        def cast_out(dst_ap, src_psum, bias_part, bias_row):
            # PSUM f32 -> SBUF bf16, optionally + bias
            if bias_part is not None:
                nc.vector.tensor_scalar_add(dst_ap, src_psum, bias_part[:])
            elif bias_row is not None:
                bcast = bass.AP(
                    tensor=bias_row.tensor,
                    offset=bias_row.offset,
                    ap=[[0, 128], [0, 4], [1, CI]],
                )
                nc.vector.tensor_tensor(dst_ap, src_psum, bcast, ALU.add)
            else:
                nc.vector.tensor_copy(dst_ap, src_psum)

        # ---- theta + first quarter of phi (enough for 8 s-tiles) ----
        with (
            tc.tile_pool(name="pj", bufs=2, space="PSUM") as pj,
            tc.tile_pool(name="wup", bufs=1, space="PSUM") as wup,
        ):
            # PE warm-up during the DMA fill: dummy matmuls flip the HAM
            # clock gate toward 8/8 before the first real matmul issues;
            # 16 of them (~5-6us) bridge until the xt/wth DMAs land
            wps = wup.tile([128, 512], F32, tag="wps")
            for _ in range(16):
                nc.tensor.matmul(
                    wps[:, 0:256], ones_sb[:], wup_rhs[:, 0:256],
                    start=True, stop=True)

            def theta_round(hh):
                tp = pj.tile([128, 1024], F32, tag="pj", name=f"tp{hh}")
                for qc in range(2):
                    for kc in range(2):
                        nc.tensor.matmul(
                            tp[:, ts(qc, 512)],
                            wth_sb[:, kc, :],
                            xt_t[hh][:, kc, ts(qc, 512)],
                            start=(kc == 0),
                            stop=(kc == 1),
                        )
                cast_out(th_sb[:, ts(hh, 1024)], tp[:], bth_sb, None)

            # theta-hh0 -> phi-hh0 -> theta-hh1, so the DVE cast chain for
            # the first exp never waits on a later DMA chunk
            theta_round(0)
            # lazy DMA issues: wph for the phi rounds, wg for the g phase,
            # xb back half for phi hh2/3 + g tiles 16..31
            nc.scalar.dma_start(
                out=wph_sb[:], in_=_kc_pair_ap(d["wphT"], CI, 0, CI))
            nc.scalar.dma_start(
                out=wg_sb[:], in_=_kc_pair_ap(d["wgT"], CI, 0, CI))
            if bg_sb is not None:
                nc.scalar.dma_start(out=bg_sb[:], in_=d["bg"][:])
            nc.sync.dma_start(
                out=xbb_sb[:], in_=_kc_pair_ap(d["xb"], N, 2048, 2048))
            pp = pj.tile([128, 1024], F32, tag="pj")
            for qc in range(2):
                for kc in range(2):
                    nc.tensor.matmul(
                        pp[:, ts(qc, 512)],
                        wph_sb[:, kc, :],
                        xbf_sb[:, kc, ts(qc, 512)],
                        start=(kc == 0),
                        stop=(kc == 1),
                    )
            cast_out(ph_sb[:, 0:1024], pp[:], bph_sb, None)
            theta_round(1)

        # ---- attention, software-pipelined against the remaining
        # projections: exp for s-tile i+8 is emitted behind the y-matmuls
        # of tile i, and the first 8 score/exp pairs precede the phi tail and
        # the whole g^T phase. phi-tail and g^T borrow the yps PSUM banks
        # (the y accumulation's start=True clears them afterwards).
        fF = [
            acts.tile([128, Q], BF16, tag=f"F{j}", name=f"F{j}")
            for j in range(4)
        ]
        ypsp_cm = tc.tile_pool(name="yps", bufs=1, space="PSUM")
        ypsp = ypsp_cm.__enter__()
        yps = ypsp.tile([128, Q], F32, tag="yps")
        scp_cm = tc.tile_pool(name="scp", bufs=2, space="PSUM")
        scp = scp_cm.__enter__()
        bp_cm = tc.tile_pool(name="bp", bufs=1)
        bp = bp_cm.__enter__()
        Bt = {}

        def sc_exp(i):
            B = bp.tile([128, Q], BF16, tag=f"B{i % N_BSLOT}", name=f"B{i}")
            Bt[i] = B
            for h in range(2):
                sc = scp.tile([128, 1024], F32, tag="sc")
                for qc in range(2):
                    nc.tensor.matmul(
                        sc[:, ts(qc, 512)],
                        ph_sb[:, ts(i, 128)],
                        th_sb[:, ts(h * 2 + qc, 512)],
                        start=True,
                        stop=True,
                    )
                if _dve_exp_half(i, h):
                    # Schraudolph exp on DVE: bf16 bits of exp(SCALE*sc)
                    nc.vector.tensor_scalar(
                        B[:, ts(h, 1024)].bitcast(I16), sc[:],
                        SCH_A, SCH_B, ALU.mult, ALU.add)
                else:
                    nc.scalar.activation(
                        B[:, ts(h, 1024)], sc[:], AF.Exp, scale=SCALE)
            # stride-4 fold chains, engine per FOLD_POOL LUT. The first add
            # of a chain sums B[j] and B[j+4] directly -- no seed copies.
            # Tile 31 is NOT folded into chain 3: it becomes the single
            # final add after the chain combines, so the tail critical path
            # past the last exp is one add. DVE folds for tiles 4..7 are
            # emitted later (after the g casts) so the g casts -- which gate
            # the whole y main loop -- aren't queued behind them.
            j = i % 4
            if i < 4 or i == 31:
                pass
            elif i in FOLD_POOL:
                if i == 4:
                    nc.gpsimd.tensor_tensor(fF[j][:], Bt[0][:], Bt[4][:], ALU.add)
                else:
                    nc.gpsimd.tensor_tensor(fF[j][:], fF[j][:], Bt[i][:], ALU.add)
            elif i >= 8:
                nc.vector.tensor_tensor(fF[j][:], fF[j][:], Bt[i][:], ALU.add)

        for i in range(8):
            sc_exp(i)

        # phi tail (tiles 8..31) into borrowed yps banks
        for hh in range(1, 4):
            pp = yps[:, ts(hh % 2, 1024)]
            xbt = xbf_sb if hh < 2 else xbb_sb
            lc = (hh % 2) * 1024
            for qc in range(2):
                for kc in range(2):
                    nc.tensor.matmul(
                        pp[:, ts(qc, 512)],
                        wph_sb[:, kc, :],
                        xbt[:, kc, lc + qc * 512:lc + qc * 512 + 512],
                        start=(kc == 0),
                        stop=(kc == 1),
                    )
            cast_out(ph_sb[:, ts(hh, 1024)], pp[:], bph_sb, None)

        # g^T projection, also into borrowed yps banks
        for b in range(8):
            gp = yps[:, 512 * (b % 4):512 * (b % 4) + 512]
            for sj in range(4):
                st = b * 4 + sj
                xbt, lc = xb_at(st)
                for kc in range(2):
                    nc.tensor.matmul(
                        gp[:, ts(sj, 128)],
                        xbt[:, kc, lc:lc + 128],
                        wg_sb[:, kc, :],
                        start=(kc == 0),
                        stop=(kc == 1),
                    )
            cast_out(gT_sb[:, ts(b, 512)], gp[:], None, bg_sb)

        # deferred DVE first-adds for chains 1..3 (tiles 5,6,7), after the
        # g casts so those never wait behind exp-gated folds
        for i in (5, 6, 7):
            nc.vector.tensor_tensor(fF[i % 4][:], Bt[i % 4][:], Bt[i][:], ALU.add)

        # ---- main loop: y(i) first, then scores/exp for i+8, so the PE
        # queue head never blocks on an exp that is still in flight ----
        for i in range(32):
            B = Bt[i]
            for h in range(2):
                for qc in range(2):
                    nc.tensor.matmul(
                        yps[:, ts(h * 2 + qc, 512)],
                        gT_sb[:, ts(i, 128)],
                        B[:, ts(h * 2 + qc, 512)],
                        start=(i == 0),
                        stop=(i == 31),
                    )
            if i == 0:
                # tail-only weights, issued now so they never gate the
                # projections' DMA waits but still land long before use
                nc.sync.dma_start(out=wo_sb[:], in_=d["woT"][:])
                nc.sync.dma_start(out=id_sb[:], in_=d["ident"][:])
                if bo_sb is not None:
                    for oc in range(2):
                        nc.sync.dma_start(out=bo_sb[:, oc, :], in_=d["bo"][oc])
            if i < 24:
                sc_exp(i + 8)
            # pre-emit the chain combines so only one add remains after
            # the last exp: c1 = F1+F0 right after fold(29) (GPSIMD chain
            # 0 ended at 28), c2 = F2+F3 and c3 after fold(30)
            if i == 21:
                nc.vector.tensor_tensor(fF[1][:], fF[1][:], fF[0][:], ALU.add)
            elif i == 22:
                nc.vector.tensor_tensor(fF[2][:], fF[2][:], fF[3][:], ALU.add)
                nc.vector.tensor_tensor(fF[1][:], fF[1][:], fF[2][:], ALU.add)
        # the single final add of tile 31's B right after its exp lands
        nc.vector.tensor_tensor(fF[1][:], fF[1][:], Bt[31][:], ALU.add)
        bp_cm.__exit__(None, None, None)
        scp_cm.__exit__(None, None, None)

        # ---- tail, pipelined per 512-wide q-chunk:
        # d (1 MM over F) -> 1/d (approx) -> y*1/d -> {residual + out-proj
        # as one PSUM accumulation: ident@xt (start) + woT@ynt (stop)} ->
        # DMA out straight from PSUM.
        with (
            tc.tile_pool(name="dps", bufs=2, space="PSUM") as dpsp,
            tc.tile_pool(name="rps", bufs=2, space="PSUM") as rps,
        ):
            for qc in range(4):
                dp = dpsp.tile([128, 512], F32, tag="dp")
                nc.tensor.matmul(
                    dp[:], ones_sb[:], fF[1][:, ts(qc, 512)],
                    start=True, stop=True)
                rcp = outp.tile([128, 512], F32, tag="rcp")
                nc.vector.reciprocal_approx_fast(rcp[:], dp[:])
                ynt = outp.tile([128, 512], BF16, tag="ynt")
                nc.vector.tensor_tensor(
                    ynt[:], yps[:, ts(qc, 512)], rcp[:], ALU.mult)
                for oc in range(2):
                    rp = rps.tile([128, 512], F32, tag="rp")
                    xres = xt_t[qc // 2][:, oc, ts(qc % 2, 512)]
                    nc.tensor.matmul(
                        rp[:], id_sb[:], xres,
                        start=True, stop=False)
                    nc.tensor.matmul(
                        rp[:],
                        wo_sb[:, ts(oc, 128)],
                        ynt[:],
                        start=False,
                        stop=True,
                    )
                    # PSUM -> SBUF split across ACT (oc0) and DVE (oc1) so
                    # the copies run concurrently (DMA cannot read PSUM);
                    # out DMAs split across the sync and scalar queues
                    ot = outp.tile([128, 512], F32, tag=f"ot{oc}")
                    if bo_sb is not None:
                        nc.scalar.activation(
                            ot[:], rp[:], AF.Identity, bias=bo_sb[:, oc, :])
                    elif oc == 0:
                        nc.scalar.copy(ot[:], rp[:])
                    else:
                        nc.vector.tensor_copy(ot[:], rp[:])
                    [nc.sync, nc.scalar][oc].dma_start(
                        out=d["out"][oc][:, ts(qc, 512)], in_=ot[:])
        ypsp_cm.__exit__(None, None, None)
        xin_cm.__exit__(None, None, None)


def _prep_in_maps(inputs):
    bf = ml_dtypes.bfloat16
    x = np.ascontiguousarray(np.asarray(inputs["x"], dtype=np.float32))
    w_g = np.asarray(inputs["w_g"], np.float32)
    b_g = np.asarray(inputs["b_g"], np.float32)
    w_theta = np.asarray(inputs["w_theta"], np.float32)
    b_theta = np.asarray(inputs["b_theta"], np.float32)
    w_phi = np.asarray(inputs["w_phi"], np.float32)
    b_phi = np.asarray(inputs["b_phi"], np.float32)
    w_out = np.asarray(inputs["w_out"], np.float32)
    b_out = np.asarray(inputs["b_out"], np.float32)

    flags = (
        bool(np.any(b_theta)), bool(np.any(b_phi)),
        bool(np.any(b_g)), bool(np.any(b_out)),
    )
    wthT = np.ascontiguousarray(w_theta.T).astype(bf).reshape(2, 128, CI)
    wphT = np.ascontiguousarray(w_phi.T).astype(bf).reshape(2, 128, CI)
    wgT = np.ascontiguousarray(w_g.T).astype(bf).reshape(2, 128, CI)
    woT = np.ascontiguousarray(w_out.T).astype(bf)          # [CI, C]
    ident = np.eye(128, dtype=bf)

    in_maps = []
    for c in range(NCORES):
        n, qh = c // 2, c % 2
        xr = x[n].reshape(C, N)
        xbc = xr.astype(bf)
        m = {
            "xb": np.ascontiguousarray(xbc.reshape(2, 128, N)),
            "xt": np.ascontiguousarray(
                xbc[:, qh * Q:(qh + 1) * Q].reshape(2, 128, Q)),
            "wthT": wthT, "wphT": wphT, "wgT": wgT, "woT": woT,
            "ident": ident,
        }
        if flags[0]:
            m["bth"] = np.ascontiguousarray(b_theta.reshape(128, 1))
        if flags[1]:
            m["bph"] = np.ascontiguousarray(b_phi.reshape(128, 1))
        if flags[2]:
            m["bg"] = np.ascontiguousarray(b_g.reshape(1, CI))
        if flags[3]:
            m["bo"] = np.ascontiguousarray(b_out.reshape(2, 128, 1))
        in_maps.append(m)
    return flags, in_maps


def _get_nc(flags):
    if flags not in _CACHE:
        _CACHE[flags] = _build(flags)
    return _CACHE[flags]


def kernel(**inputs):
    flags, in_maps = _prep_in_maps(inputs)
    nc = _get_nc(flags)
    res = run_bass_kernel_spmd(nc, in_maps, list(range(NCORES)))
    out = np.empty((NB, C, N), np.float32)
    for c in range(NCORES):
        n, qh = c // 2, c % 2
        out[n][:, qh * Q:(qh + 1) * Q] = res.results[c]["out"].reshape(C, Q)
    return out.reshape(NB, C, 64, 64)


if __name__ == "__main__":
    x = np.random.randn(NB, C, 64, 64).astype(np.float32) * 0.1
    rng = np.random.default_rng(0)
    ins = {
        "x": x,
        "w_g": rng.normal(size=(CI, C)).astype(np.float32) * 0.01,
        "b_g": np.zeros(CI, np.float32),
        "w_theta": rng.normal(size=(CI, C)).astype(np.float32) * 0.01,
        "b_theta": np.zeros(CI, np.float32),
        "w_phi": rng.normal(size=(CI, C)).astype(np.float32) * 0.01,
        "b_phi": np.zeros(CI, np.float32),
        "w_out": rng.normal(size=(C, CI)).astype(np.float32) * 0.01,
        "b_out": np.zeros(C, np.float32),
    }
    o = kernel(**ins)
    print("ok", o.shape, o.dtype)


# revision 33
# speedup vs baseline: 1.1885x; 1.0114x over previous
"""NonLocal2D (attention) block on 8 trn2 NeuronCores.

Sharding: core c -> batch n = c//2, query-half qh = c%2 (2048 of the 4096
spatial positions). Each core receives the full x[n] (so phi/g are computed
locally -- no collectives) plus its own query slice, and produces
out[n][:, qh*2048:(qh+1)*2048].

Per-core dataflow (layouts chosen so no transposes are ever needed):
  theta:    [CI=128, Q]  = wthT-chunks (lhsT) @ xt-chunks (rhs)     [PE]
  phi:      [CI=128, N]  = wphT-chunks (lhsT) @ xb-chunks (rhs)     [PE]
  g^T:      [s, CI] tiles = xb-chunks (lhsT) @ wgT-chunks (rhs)     [PE]
  scores^T: [s=128, q=1024] = phi-tile (lhsT) @ theta (rhs)         [PE -> PSUM f32]
  B = exp(SCALE*scores^T) -> bf16 SBUF; most half-tiles on ACT, a
      subset on DVE via the Schraudolph int16-bits trick (bf16 bits =
      trunc(x*128/ln2 + 127*128 - c))  (no max-sub: |scaled| < ~30)  [ACT+DVE]
  denom: 4 stride-4 fold chains over B tiles (split GPSIMD/DVE via a
      static LUT; the first add of each chain combines two B tiles so
      no copies are needed), combined to one F, then
      d = ones (lhsT) @ F per q-chunk                               [DVE/GPSIMD/PE]
  y^T += gT-tile (lhsT) @ B   (PSUM accumulate over 32 s-tiles)     [PE]
  y_norm^T = y^T * (1/d) -> bf16                                    [DVE]
  out-proj + residual in one PSUM group:
      rp = ident (lhsT) @ xt-chunk  (start)  -- the +x residual
      rp += woT-chunk (lhsT) @ y_norm^T (stop); DMA out from PSUM   [PE]

The residual uses the bf16 xt copy (no separate f32 x load): the extra
~2^-9 relative rounding on x costs ~1.7e-3 rel err, far under the 2e-2
gate, and saves 4MB/core of input DMA.
"""

import numpy as np
import ml_dtypes

import concourse.bass as bass
import concourse.mybir as mybir
import concourse.tile as tile
from concourse import bacc
from concourse.bass import ts
from concourse.bass_utils import run_bass_kernel_spmd

BF16 = mybir.dt.bfloat16
F32 = mybir.dt.float32
I16 = mybir.dt.int16
AF = mybir.ActivationFunctionType
ALU = mybir.AluOpType

C = 256          # in channels
CI = 128         # inter channels
NB = 4           # batch
N = 4096         # H*W
Q = 2048         # queries per core
NCORES = 8
SCALE = float(128 ** 0.5)   # reference divides by d**-0.5

# Schraudolph exp in bf16-bit space: bits = trunc(x*SCALE*128/ln2 + b)
SCH_A = SCALE * 128.0 / float(np.log(2.0))
SCH_B = 127.0 * 128.0 - 5.0

N_BSLOT = 24     # B-tile ring (WAR distance 24 >> pipeline depth)

# fold engine LUT: which tiles' fold-adds run on GPSIMD (rest on DVE).
# Exactly chain 0 (j = i%4 == 0): a whole chain per engine, so no
# cross-engine handoff ever head-blocks the DVE queue.
FOLD_POOL = {4, 8, 12, 16, 20, 24, 28}

_CACHE: dict = {}


def _dve_exp_half(i, h):
    # which exp half-tiles run on DVE (Schraudolph) instead of ACT.
    # Tiles 30/31 stay on ACT: at that point DVE is running the fold
    # combines that gate the tail, and the last exp must not queue there.
    # 29 goes to DVE so ACT reaches the final exps sooner.
    return h == 0 and (i % 3 == 1 or i == 29) and i < 30


def _build(flags):
    bth_nz, bph_nz, bg_nz, bo_nz = flags
    nc = bacc.Bacc("TRN2", target_bir_lowering=False, debug=False)

    d = {}
    d["xb"] = nc.dram_tensor("xb", [2, 128, N], BF16, kind="ExternalInput").ap()
    d["xt"] = nc.dram_tensor("xt", [2, 128, Q], BF16, kind="ExternalInput").ap()
    d["wthT"] = nc.dram_tensor("wthT", [2, 128, CI], BF16, kind="ExternalInput").ap()
    d["wphT"] = nc.dram_tensor("wphT", [2, 128, CI], BF16, kind="ExternalInput").ap()
    d["wgT"] = nc.dram_tensor("wgT", [2, 128, CI], BF16, kind="ExternalInput").ap()
    d["woT"] = nc.dram_tensor("woT", [128, C], BF16, kind="ExternalInput").ap()
    d["ident"] = nc.dram_tensor("ident", [128, 128], BF16, kind="ExternalInput").ap()
    d["bth"] = nc.dram_tensor("bth", [128, 1], F32, kind="ExternalInput").ap() if bth_nz else None
    d["bph"] = nc.dram_tensor("bph", [128, 1], F32, kind="ExternalInput").ap() if bph_nz else None
    d["bg"] = nc.dram_tensor("bg", [1, CI], F32, kind="ExternalInput").ap() if bg_nz else None
    d["bo"] = nc.dram_tensor("bo", [2, 128, 1], F32, kind="ExternalInput").ap() if bo_nz else None
    d["out"] = nc.dram_tensor("out", [2, 128, Q], F32, kind="ExternalOutput").ap()

    with tile.TileContext(nc) as tc:
        _bass_body(tc, d)
    nc.compile()
    return nc


def _kc_pair_ap(dram_ap, cols, col0, count):
    """3D dram AP reading [2,128,cols] as [p=128, kc=2, count] at col0."""
    return bass.AP(
        tensor=dram_ap.tensor,
        offset=col0,
        ap=[[cols, 128], [128 * cols, 2], [1, count]],
    )


def _bass_body(tc, d):
    nc = tc.nc

    with (
        tc.tile_pool(name="const", bufs=1) as const,
        tc.tile_pool(name="acts", bufs=1) as acts,
        tc.tile_pool(name="outs", bufs=2) as outp,
    ):
        # ---- constants / weights ----
        # memsets on DVE (idle at t=0) so the PE warm-up never waits on the
        # gpsimd program load; the exp-table warm reads scratch itself
        # (garbage in, table warmed) so it carries no cross-engine dep
        ones_sb = const.tile([128, 128], BF16, tag="ones")
        nc.vector.memset(ones_sb[:], 1.0)
        wup_rhs = const.tile([128, 512], BF16, tag="wup_rhs")
        nc.vector.memset(wup_rhs[:], 0.0)
        scratch = const.tile([128, 1], BF16, tag="scratch")
        nc.vector.memset(scratch[:], 1.0)

        wth_sb = const.tile([128, 2, CI], BF16, tag="wth")
        wph_sb = const.tile([128, 2, CI], BF16, tag="wph")
        wg_sb = const.tile([128, 2, CI], BF16, tag="wg")
        wo_sb = const.tile([128, C], BF16, tag="wo")
        id_sb = const.tile([128, 128], BF16, tag="ident")
        bth_sb = bph_sb = bg_sb = bo_sb = None
        if d["bth"] is not None:
            bth_sb = const.tile([128, 1], F32, tag="bth")
        if d["bph"] is not None:
            bph_sb = const.tile([128, 1], F32, tag="bph")
        if d["bg"] is not None:
            bg_sb = const.tile([1, CI], F32, tag="bg")
        if d["bo"] is not None:
            bo_sb = const.tile([128, 2, 1], F32, tag="bo")

        th_sb = acts.tile([128, Q], BF16, tag="th")
        ph_sb = acts.tile([128, N], BF16, tag="ph")
        gT_sb = acts.tile([128, 32 * CI], BF16, tag="gT")  # tile i at cols [128i, 128i+128)

        # ---- input fill: one tile per DMA transfer so every consumer's
        # RAW dep is exactly one transfer (no waiting on sibling chunks),
        # ordered by first use across three queues ----
        xin_cm = tc.tile_pool(name="xin", bufs=1)
        xin = xin_cm.__enter__()
        xt_t = [
            xin.tile([128, 2, 1024], BF16, tag=f"xt{hh}", name=f"xt{hh}")
            for hh in range(2)
        ]
        xbf_sb = xin.tile([128, 2, 2048], BF16, tag="xbf")
        xbb_sb = xin.tile([128, 2, 2048], BF16, tag="xbb")

        # Only the transfers needed FIRST are issued here. DMA waits appear
        # to coalesce to "every transfer issued so far on that queue", so
        # later-needed tensors (wg, wo, ident, xb back half, biases) are
        # issued lazily at their point of need to keep theta's wait minimal.
        # sync: xt halves (theta); scalar: wth; gpsimd: front half of xb
        for hh in range(2):
            nc.sync.dma_start(
                out=xt_t[hh][:],
                in_=_kc_pair_ap(d["xt"], Q, hh * 1024, 1024))
        nc.scalar.dma_start(out=wth_sb[:], in_=_kc_pair_ap(d["wthT"], CI, 0, CI))
        if bth_sb is not None:
            nc.scalar.dma_start(out=bth_sb[:], in_=d["bth"][:])
        if bph_sb is not None:
            nc.scalar.dma_start(out=bph_sb[:], in_=d["bph"][:])
        nc.gpsimd.dma_start(
            out=xbf_sb[:], in_=_kc_pair_ap(d["xb"], N, 0, 2048))

        def xb_at(st128):
            # (tile, local col0) for xb column st128*128
            col = st128 * 128
            return (xbf_sb, col) if col < 2048 else (xbb_sb, col - 2048)

        # warm the exp table set early so the first real exp isn't +2.7us
        nc.scalar.activation(scratch[:], scratch[:], AF.Exp, scale=1.0)


        def cast_out(dst_ap, src_psum, bias_part, bias_row):
            # PSUM f32 -> SBUF bf16, optionally + bias
            if bias_part is not None:
                nc.vector.tensor_scalar_add(dst_ap, src_psum, bias_part[:])
            elif bias_row is not None:
                bcast = bass.AP(
                    tensor=bias_row.tensor,
                    offset=bias_row.offset,
                    ap=[[0, 128], [0, 4], [1, CI]],
                )
                nc.vector.tensor_tensor(dst_ap, src_psum, bcast, ALU.add)
            else:
                nc.vector.tensor_copy(dst_ap, src_psum)

        # ---- theta + first quarter of phi (enough for 8 s-tiles) ----
        with (
            tc.tile_pool(name="pj", bufs=2, space="PSUM") as pj,
            tc.tile_pool(name="wup", bufs=1, space="PSUM") as wup,
        ):
            # PE warm-up during the DMA fill: dummy matmuls flip the HAM
            # clock gate toward 8/8 before the first real matmul issues.
            # 8 of them (~4us at the cold clock) end right as the first
            # xt/wth transfers land -- more would push theta out.
            wps = wup.tile([128, 512], F32, tag="wps")
            for _ in range(8):
                nc.tensor.matmul(
                    wps[:, 0:256], ones_sb[:], wup_rhs[:, 0:256],
                    start=True, stop=True)

            def theta_round(hh):
                tp = pj.tile([128, 1024], F32, tag="pj", name=f"tp{hh}")
                for qc in range(2):
                    for kc in range(2):
                        nc.tensor.matmul(
                            tp[:, ts(qc, 512)],
                            wth_sb[:, kc, :],
                            xt_t[hh][:, kc, ts(qc, 512)],
                            start=(kc == 0),
                            stop=(kc == 1),
                        )
                cast_out(th_sb[:, ts(hh, 1024)], tp[:], bth_sb, None)

            # theta-hh0 -> phi-hh0 -> theta-hh1, so the DVE cast chain for
            # the first exp never waits on a later DMA chunk
            theta_round(0)
            # lazy DMA issues: wph for the phi rounds, wg for the g phase,
            # xb back half for phi hh2/3 + g tiles 16..31
            nc.scalar.dma_start(
                out=wph_sb[:], in_=_kc_pair_ap(d["wphT"], CI, 0, CI))
            nc.scalar.dma_start(
                out=wg_sb[:], in_=_kc_pair_ap(d["wgT"], CI, 0, CI))
            if bg_sb is not None:
                nc.scalar.dma_start(out=bg_sb[:], in_=d["bg"][:])
            nc.sync.dma_start(
                out=xbb_sb[:], in_=_kc_pair_ap(d["xb"], N, 2048, 2048))
            pp = pj.tile([128, 1024], F32, tag="pj")
            for qc in range(2):
                for kc in range(2):
                    nc.tensor.matmul(
                        pp[:, ts(qc, 512)],
                        wph_sb[:, kc, :],
                        xbf_sb[:, kc, ts(qc, 512)],
                        start=(kc == 0),
                        stop=(kc == 1),
                    )
            cast_out(ph_sb[:, 0:1024], pp[:], bph_sb, None)
            theta_round(1)

        # ---- attention, software-pipelined against the remaining
        # projections: exp for s-tile i+8 is emitted behind the y-matmuls
        # of tile i, and the first 8 score/exp pairs precede the phi tail and
        # the whole g^T phase. phi-tail and g^T borrow the yps PSUM banks
        # (the y accumulation's start=True clears them afterwards).
        fF = [
            acts.tile([128, Q], BF16, tag=f"F{j}", name=f"F{j}")
            for j in range(4)
        ]
        ypsp_cm = tc.tile_pool(name="yps", bufs=1, space="PSUM")
        ypsp = ypsp_cm.__enter__()
        yps = ypsp.tile([128, Q], F32, tag="yps")
        scp_cm = tc.tile_pool(name="scp", bufs=2, space="PSUM")
        scp = scp_cm.__enter__()
        bp_cm = tc.tile_pool(name="bp", bufs=1)
        bp = bp_cm.__enter__()
        Bt = {}

        def sc_exp(i):
            B = bp.tile([128, Q], BF16, tag=f"B{i % N_BSLOT}", name=f"B{i}")
            Bt[i] = B
            for h in range(2):
                sc = scp.tile([128, 1024], F32, tag="sc")
                for qc in range(2):
                    nc.tensor.matmul(
                        sc[:, ts(qc, 512)],
                        ph_sb[:, ts(i, 128)],
                        th_sb[:, ts(h * 2 + qc, 512)],
                        start=True,
                        stop=True,
                    )
                if _dve_exp_half(i, h):
                    # Schraudolph exp on DVE: bf16 bits of exp(SCALE*sc)
                    nc.vector.tensor_scalar(
                        B[:, ts(h, 1024)].bitcast(I16), sc[:],
                        SCH_A, SCH_B, ALU.mult, ALU.add)
                else:
                    nc.scalar.activation(
                        B[:, ts(h, 1024)], sc[:], AF.Exp, scale=SCALE)
            # stride-4 fold chains, engine per FOLD_POOL LUT. The first add
            # of a chain sums B[j] and B[j+4] directly -- no seed copies.
            # Tile 31 is NOT folded into chain 3: it becomes the single
            # final add after the chain combines, so the tail critical path
            # past the last exp is one add. DVE folds for tiles 4..7 are
            # emitted later (after the g casts) so the g casts -- which gate
            # the whole y main loop -- aren't queued behind them.
            j = i % 4
            if i < 4 or i == 31:
                pass
            elif i in FOLD_POOL:
                if i == 4:
                    nc.gpsimd.tensor_tensor(fF[j][:], Bt[0][:], Bt[4][:], ALU.add)
                else:
                    nc.gpsimd.tensor_tensor(fF[j][:], fF[j][:], Bt[i][:], ALU.add)
            elif i >= 8:
                nc.vector.tensor_tensor(fF[j][:], fF[j][:], Bt[i][:], ALU.add)

        for i in range(8):
            sc_exp(i)

        # phi tail (tiles 8..31) into borrowed yps banks
        for hh in range(1, 4):
            pp = yps[:, ts(hh % 2, 1024)]
            xbt = xbf_sb if hh < 2 else xbb_sb
            lc = (hh % 2) * 1024
            for qc in range(2):
                for kc in range(2):
                    nc.tensor.matmul(
                        pp[:, ts(qc, 512)],
                        wph_sb[:, kc, :],
                        xbt[:, kc, lc + qc * 512:lc + qc * 512 + 512],
                        start=(kc == 0),
                        stop=(kc == 1),
                    )
            cast_out(ph_sb[:, ts(hh, 1024)], pp[:], bph_sb, None)

        # g^T projection, also into borrowed yps banks. The casts split
        # between DVE and ACT: the y main loop can only begin once ALL g
        # casts have drained the borrowed banks, and DVE alone finishes
        # ~2us after the g matmuls do.
        for b in range(8):
            gp = yps[:, 512 * (b % 4):512 * (b % 4) + 512]
            for sj in range(4):
                st = b * 4 + sj
                xbt, lc = xb_at(st)
                for kc in range(2):
                    nc.tensor.matmul(
                        gp[:, ts(sj, 128)],
                        xbt[:, kc, lc:lc + 128],
                        wg_sb[:, kc, :],
                        start=(kc == 0),
                        stop=(kc == 1),
                    )
            if bg_sb is None and b % 2 == 1:
                nc.scalar.copy(gT_sb[:, ts(b, 512)], gp[:])
            else:
                cast_out(gT_sb[:, ts(b, 512)], gp[:], None, bg_sb)

        # deferred DVE first-adds for chains 1..3 (tiles 5,6,7), after the
        # g casts so those never wait behind exp-gated folds
        for i in (5, 6, 7):
            nc.vector.tensor_tensor(fF[i % 4][:], Bt[i % 4][:], Bt[i][:], ALU.add)

        # ---- main loop: y(i) first, then scores/exp for i+8, so the PE
        # queue head never blocks on an exp that is still in flight ----
        for i in range(32):
            B = Bt[i]
            for h in range(2):
                for qc in range(2):
                    nc.tensor.matmul(
                        yps[:, ts(h * 2 + qc, 512)],
                        gT_sb[:, ts(i, 128)],
                        B[:, ts(h * 2 + qc, 512)],
                        start=(i == 0),
                        stop=(i == 31),
                    )
            if i == 0:
                # tail-only weights, issued now so they never gate the
                # projections' DMA waits but still land long before use
                nc.sync.dma_start(out=wo_sb[:], in_=d["woT"][:])
                nc.sync.dma_start(out=id_sb[:], in_=d["ident"][:])
                if bo_sb is not None:
                    for oc in range(2):
                        nc.sync.dma_start(out=bo_sb[:, oc, :], in_=d["bo"][oc])
            if i < 24:
                sc_exp(i + 8)
            # pre-emit the chain combines so only one add remains after
            # the last exp: c1 = F1+F0 right after fold(29) (GPSIMD chain
            # 0 ended at 28), c2 = F2+F3 and c3 after fold(30)
            if i == 21:
                nc.vector.tensor_tensor(fF[1][:], fF[1][:], fF[0][:], ALU.add)
            elif i == 22:
                nc.vector.tensor_tensor(fF[2][:], fF[2][:], fF[3][:], ALU.add)
                nc.vector.tensor_tensor(fF[1][:], fF[1][:], fF[2][:], ALU.add)
        # final add of tile 31's B, in two halves: the first half only
        # needs exp(31,h0), so the qc0/qc1 denominators start while
        # exp(31,h1) is still on ACT
        for hh in range(2):
            nc.vector.tensor_tensor(
                fF[1][:, ts(hh, 1024)], fF[1][:, ts(hh, 1024)],
                Bt[31][:, ts(hh, 1024)], ALU.add)
        bp_cm.__exit__(None, None, None)
        scp_cm.__exit__(None, None, None)

        # ---- tail, pipelined per 512-wide q-chunk:
        # d (1 MM over F) -> 1/d (approx) -> y*1/d -> {residual + out-proj
        # as one PSUM accumulation: ident@xt (start) + woT@ynt (stop)} ->
        # DMA out straight from PSUM.
        with (
            tc.tile_pool(name="dps", bufs=2, space="PSUM") as dpsp,
            tc.tile_pool(name="rps", bufs=2, space="PSUM") as rps,
        ):
            for qc in range(4):
                dp = dpsp.tile([128, 512], F32, tag="dp")
                nc.tensor.matmul(
                    dp[:], ones_sb[:], fF[1][:, ts(qc, 512)],
                    start=True, stop=True)
                rcp = outp.tile([128, 512], F32, tag="rcp")
                nc.vector.reciprocal_approx_fast(rcp[:], dp[:])
                ynt = outp.tile([128, 512], BF16, tag="ynt")
                nc.vector.tensor_tensor(
                    ynt[:], yps[:, ts(qc, 512)], rcp[:], ALU.mult)
                for oc in range(2):
                    rp = rps.tile([128, 512], F32, tag="rp")
                    xres = xt_t[qc // 2][:, oc, ts(qc % 2, 512)]
                    nc.tensor.matmul(
                        rp[:], id_sb[:], xres,
                        start=True, stop=False)
                    nc.tensor.matmul(
                        rp[:],
                        wo_sb[:, ts(oc, 128)],
                        ynt[:],
                        start=False,
                        stop=True,
                    )
                    # PSUM -> SBUF split across ACT (oc0) and DVE (oc1) so
                    # the copies run concurrently (DMA cannot read PSUM);
                    # out DMAs split across the sync and scalar queues
                    ot = outp.tile([128, 512], F32, tag=f"ot{oc}")
                    if bo_sb is not None:
                        nc.scalar.activation(
                            ot[:], rp[:], AF.Identity, bias=bo_sb[:, oc, :])
                    elif oc == 0:
                        nc.scalar.copy(ot[:], rp[:])
                    else:
                        nc.vector.tensor_copy(ot[:], rp[:])
                    [nc.sync, nc.scalar][oc].dma_start(
                        out=d["out"][oc][:, ts(qc, 512)], in_=ot[:])
        ypsp_cm.__exit__(None, None, None)
        xin_cm.__exit__(None, None, None)


def _prep_in_maps(inputs):
    bf = ml_dtypes.bfloat16
    x = np.ascontiguousarray(np.asarray(inputs["x"], dtype=np.float32))
    w_g = np.asarray(inputs["w_g"], np.float32)
    b_g = np.asarray(inputs["b_g"], np.float32)
    w_theta = np.asarray(inputs["w_theta"], np.float32)
    b_theta = np.asarray(inputs["b_theta"], np.float32)
    w_phi = np.asarray(inputs["w_phi"], np.float32)
    b_phi = np.asarray(inputs["b_phi"], np.float32)
    w_out = np.asarray(inputs["w_out"], np.float32)
    b_out = np.asarray(inputs["b_out"], np.float32)

    flags = (
        bool(np.any(b_theta)), bool(np.any(b_phi)),
        bool(np.any(b_g)), bool(np.any(b_out)),
    )
    wthT = np.ascontiguousarray(w_theta.T).astype(bf).reshape(2, 128, CI)
    wphT = np.ascontiguousarray(w_phi.T).astype(bf).reshape(2, 128, CI)
    wgT = np.ascontiguousarray(w_g.T).astype(bf).reshape(2, 128, CI)
    woT = np.ascontiguousarray(w_out.T).astype(bf)          # [CI, C]
    ident = np.eye(128, dtype=bf)

    in_maps = []
    for c in range(NCORES):
        n, qh = c // 2, c % 2
        xr = x[n].reshape(C, N)
        xbc = xr.astype(bf)
        m = {
            "xb": np.ascontiguousarray(xbc.reshape(2, 128, N)),
            "xt": np.ascontiguousarray(
                xbc[:, qh * Q:(qh + 1) * Q].reshape(2, 128, Q)),
            "wthT": wthT, "wphT": wphT, "wgT": wgT, "woT": woT,
            "ident": ident,
        }
        if flags[0]:
            m["bth"] = np.ascontiguousarray(b_theta.reshape(128, 1))
        if flags[1]:
            m["bph"] = np.ascontiguousarray(b_phi.reshape(128, 1))
        if flags[2]:
            m["bg"] = np.ascontiguousarray(b_g.reshape(1, CI))
        if flags[3]:
            m["bo"] = np.ascontiguousarray(b_out.reshape(2, 128, 1))
        in_maps.append(m)
    return flags, in_maps


def _get_nc(flags):
    if flags not in _CACHE:
        _CACHE[flags] = _build(flags)
    return _CACHE[flags]


def kernel(**inputs):
    flags, in_maps = _prep_in_maps(inputs)
    nc = _get_nc(flags)
    res = run_bass_kernel_spmd(nc, in_maps, list(range(NCORES)))
    out = np.empty((NB, C, N), np.float32)
    for c in range(NCORES):
        n, qh = c // 2, c % 2
        out[n][:, qh * Q:(qh + 1) * Q] = res.results[c]["out"].reshape(C, Q)
    return out.reshape(NB, C, 64, 64)


if __name__ == "__main__":
    x = np.random.randn(NB, C, 64, 64).astype(np.float32) * 0.1
    rng = np.random.default_rng(0)
    ins = {
        "x": x,
        "w_g": rng.normal(size=(CI, C)).astype(np.float32) * 0.01,
        "b_g": np.zeros(CI, np.float32),
        "w_theta": rng.normal(size=(CI, C)).astype(np.float32) * 0.01,
        "b_theta": np.zeros(CI, np.float32),
        "w_phi": rng.normal(size=(CI, C)).astype(np.float32) * 0.01,
        "b_phi": np.zeros(CI, np.float32),
        "w_out": rng.normal(size=(C, CI)).astype(np.float32) * 0.01,
        "b_out": np.zeros(C, np.float32),
    }
    o = kernel(**ins)
    print("ok", o.shape, o.dtype)


# revision 40
# speedup vs baseline: 1.2179x; 1.0247x over previous
"""NonLocal2D (attention) block on 8 trn2 NeuronCores.

Sharding: core c -> batch n = c//2, query-half qh = c%2 (2048 of the 4096
spatial positions). Each core receives the full x[n] (so phi/g are computed
locally -- no collectives) plus its own query slice, and produces
out[n][:, qh*2048:(qh+1)*2048].

Per-core dataflow (layouts chosen so no transposes are ever needed):
  theta:    [CI=128, Q]  = wthT-chunks (lhsT) @ xt-chunks (rhs)     [PE]
  phi:      [CI=128, N]  = wphT-chunks (lhsT) @ xb-chunks (rhs)     [PE]
  g^T:      [s, CI] tiles = xb-chunks (lhsT) @ wgT-chunks (rhs)     [PE]
  scores^T: [s=128, q=1024] = phi-tile (lhsT) @ theta (rhs)         [PE -> PSUM f32]
  B = exp(SCALE*scores^T) -> bf16 SBUF; most half-tiles on ACT, a
      subset on DVE via the Schraudolph int16-bits trick (bf16 bits =
      trunc(x*128/ln2 + 127*128 - c))  (no max-sub: |scaled| < ~30)  [ACT+DVE]
  denom: 4 stride-4 fold chains over B tiles (split GPSIMD/DVE via a
      static LUT; the first add of each chain combines two B tiles so
      no copies are needed), combined to one F, then
      d = ones (lhsT) @ F per q-chunk                               [DVE/GPSIMD/PE]
  y^T += gT-tile (lhsT) @ B   (PSUM accumulate over 32 s-tiles)     [PE]
  y_norm^T = y^T * (1/d) -> bf16                                    [DVE]
  out-proj + residual in one PSUM group:
      rp = ident (lhsT) @ xt-chunk  (start)  -- the +x residual
      rp += woT-chunk (lhsT) @ y_norm^T (stop); DMA out from PSUM   [PE]

The residual uses the bf16 xt copy (no separate f32 x load): the extra
~2^-9 relative rounding on x costs ~1.7e-3 rel err, far under the 2e-2
gate, and saves 4MB/core of input DMA.
"""

import numpy as np
import ml_dtypes

import concourse.bass as bass
import concourse.mybir as mybir
import concourse.tile as tile
from concourse import bacc
from concourse.bass import ts
from concourse.bass_utils import run_bass_kernel_spmd

BF16 = mybir.dt.bfloat16
F32 = mybir.dt.float32
I16 = mybir.dt.int16
AF = mybir.ActivationFunctionType
ALU = mybir.AluOpType

C = 256          # in channels
CI = 128         # inter channels
NB = 4           # batch
N = 4096         # H*W
Q = 2048         # queries per core
NCORES = 8
SCALE = float(128 ** 0.5)   # reference divides by d**-0.5

# Schraudolph exp in bf16-bit space: bits = trunc(x*SCALE*128/ln2 + b)
SCH_A = SCALE * 128.0 / float(np.log(2.0))
SCH_B = 127.0 * 128.0 - 5.0

N_BSLOT = 24     # B-tile ring (WAR distance 24 >> pipeline depth)

# fold engine LUT: which tiles' fold-adds run on GPSIMD (rest on DVE).
# Exactly chain 0 (j = i%4 == 0): a whole chain per engine, so no
# cross-engine handoff ever head-blocks the DVE queue.
FOLD_POOL = {4, 8, 12, 16, 20, 24, 28}

_CACHE: dict = {}


def _dve_exp_half(i, h):
    # which exp half-tiles run on DVE (Schraudolph) instead of ACT.
    # Tiles 30/31 stay fully on ACT (the last exps gate the tail and DVE
    # still owes their folds); both halves of 29 go to DVE so ACT
    # reaches the final exps sooner.
    return (h == 0 and i % 3 == 1 and i < 29) or i == 29


def _build(flags):
    bth_nz, bph_nz, bg_nz, bo_nz = flags
    nc = bacc.Bacc("TRN2", target_bir_lowering=False, debug=False)

    d = {}
    d["xb"] = nc.dram_tensor("xb", [2, 128, N], BF16, kind="ExternalInput").ap()
    d["xt"] = nc.dram_tensor("xt", [2, 128, Q], BF16, kind="ExternalInput").ap()
    d["wthT"] = nc.dram_tensor("wthT", [2, 128, CI], BF16, kind="ExternalInput").ap()
    d["wphT"] = nc.dram_tensor("wphT", [2, 128, CI], BF16, kind="ExternalInput").ap()
    d["wgT"] = nc.dram_tensor("wgT", [2, 128, CI], BF16, kind="ExternalInput").ap()
    d["woT"] = nc.dram_tensor("woT", [128, C], BF16, kind="ExternalInput").ap()
    d["ident"] = nc.dram_tensor("ident", [128, 128], BF16, kind="ExternalInput").ap()
    d["bth"] = nc.dram_tensor("bth", [128, 1], F32, kind="ExternalInput").ap() if bth_nz else None
    d["bph"] = nc.dram_tensor("bph", [128, 1], F32, kind="ExternalInput").ap() if bph_nz else None
    d["bg"] = nc.dram_tensor("bg", [1, CI], F32, kind="ExternalInput").ap() if bg_nz else None
    d["bo"] = nc.dram_tensor("bo", [2, 128, 1], F32, kind="ExternalInput").ap() if bo_nz else None
    d["out"] = nc.dram_tensor("out", [2, 128, Q], F32, kind="ExternalOutput").ap()

    with tile.TileContext(nc) as tc:
        _bass_body(tc, d)
    nc.compile()
    return nc


def _kc_pair_ap(dram_ap, cols, col0, count):
    """3D dram AP reading [2,128,cols] as [p=128, kc=2, count] at col0."""
    return bass.AP(
        tensor=dram_ap.tensor,
        offset=col0,
        ap=[[cols, 128], [128 * cols, 2], [1, count]],
    )


def _bass_body(tc, d):
    nc = tc.nc

    with (
        tc.tile_pool(name="const", bufs=1) as const,
        tc.tile_pool(name="acts", bufs=1) as acts,
        tc.tile_pool(name="outs", bufs=2) as outp,
    ):
        # ---- constants / weights ----
        # memsets on DVE (idle at t=0) so the PE warm-up never waits on the
        # gpsimd program load; the exp-table warm reads scratch itself
        # (garbage in, table warmed) so it carries no cross-engine dep
        ones_sb = const.tile([128, 128], BF16, tag="ones")
        nc.vector.memset(ones_sb[:], 1.0)
        wup_rhs = const.tile([128, 512], BF16, tag="wup_rhs")
        nc.vector.memset(wup_rhs[:], 0.0)
        scratch = const.tile([128, 1], BF16, tag="scratch")
        nc.vector.memset(scratch[:], 1.0)

        wth_sb = const.tile([128, 2, CI], BF16, tag="wth")
        wph_sb = const.tile([128, 2, CI], BF16, tag="wph")
        wg_sb = const.tile([128, 2, CI], BF16, tag="wg")
        wo_sb = const.tile([128, C], BF16, tag="wo")
        id_sb = const.tile([128, 128], BF16, tag="ident")
        bth_sb = bph_sb = bg_sb = bo_sb = None
        if d["bth"] is not None:
            bth_sb = const.tile([128, 1], F32, tag="bth")
        if d["bph"] is not None:
            bph_sb = const.tile([128, 1], F32, tag="bph")
        if d["bg"] is not None:
            bg_sb = const.tile([1, CI], F32, tag="bg")
        if d["bo"] is not None:
            bo_sb = const.tile([128, 2, 1], F32, tag="bo")

        th_sb = acts.tile([128, Q], BF16, tag="th")
        ph_sb = acts.tile([128, N], BF16, tag="ph")
        gT_sb = acts.tile([128, 32 * CI], BF16, tag="gT")  # tile i at cols [128i, 128i+128)

        # ---- input fill: one tile per DMA transfer so every consumer's
        # RAW dep is exactly one transfer (no waiting on sibling chunks),
        # ordered by first use across three queues ----
        xin_cm = tc.tile_pool(name="xin", bufs=1)
        xin = xin_cm.__enter__()
        xt_t = [
            xin.tile([128, 2, 1024], BF16, tag=f"xt{hh}", name=f"xt{hh}")
            for hh in range(2)
        ]
        xbf_t = [
            xin.tile([128, 2, 1024], BF16, tag=f"xbf{c}", name=f"xbf{c}")
            for c in range(2)
        ]
        xbb_sb = xin.tile([128, 2, 2048], BF16, tag="xbb")

        # Only the transfers needed FIRST are issued here. DMA waits appear
        # to coalesce to "every transfer issued so far on that queue", so
        # later-needed tensors (wg, wo, ident, xb back half, biases) are
        # issued lazily at their point of need to keep theta's wait minimal.
        # sync: xt halves (theta); scalar: wth; gpsimd: front half of xb
        for hh in range(2):
            nc.sync.dma_start(
                out=xt_t[hh][:],
                in_=_kc_pair_ap(d["xt"], Q, hh * 1024, 1024))
        nc.scalar.dma_start(out=wth_sb[:], in_=_kc_pair_ap(d["wthT"], CI, 0, CI))
        if bth_sb is not None:
            nc.scalar.dma_start(out=bth_sb[:], in_=d["bth"][:])
        if bph_sb is not None:
            nc.scalar.dma_start(out=bph_sb[:], in_=d["bph"][:])
        for c in range(2):
            nc.gpsimd.dma_start(
                out=xbf_t[c][:], in_=_kc_pair_ap(d["xb"], N, c * 1024, 1024))

        def xb_at(st128):
            # (tile, local col0) for xb column st128*128
            col = st128 * 128
            if col < 1024:
                return xbf_t[0], col
            if col < 2048:
                return xbf_t[1], col - 1024
            return xbb_sb, col - 2048

        # warm the exp table set early so the first real exp isn't +2.7us
        nc.scalar.activation(scratch[:], scratch[:], AF.Exp, scale=1.0)


        def cast_out(dst_ap, src_psum, bias_part, bias_row):
            # PSUM f32 -> SBUF bf16, optionally + bias
            if bias_part is not None:
                nc.vector.tensor_scalar_add(dst_ap, src_psum, bias_part[:])
            elif bias_row is not None:
                bcast = bass.AP(
                    tensor=bias_row.tensor,
                    offset=bias_row.offset,
                    ap=[[0, 128], [0, 4], [1, CI]],
                )
                nc.vector.tensor_tensor(dst_ap, src_psum, bcast, ALU.add)
            else:
                nc.vector.tensor_copy(dst_ap, src_psum)

        # ---- theta + first quarter of phi (enough for 8 s-tiles) ----
        with (
            tc.tile_pool(name="pj", bufs=2, space="PSUM") as pj,
            tc.tile_pool(name="wup", bufs=1, space="PSUM") as wup,
        ):
            # PE warm-up during the DMA fill: dummy matmuls flip the HAM
            # clock gate toward 8/8 before the first real matmul issues.
            # 8 of them (~4us at the cold clock) end right as the first
            # xt/wth transfers land -- more would push theta out.
            wps = wup.tile([128, 512], F32, tag="wps")
            for _ in range(8):
                nc.tensor.matmul(
                    wps[:, 0:256], ones_sb[:], wup_rhs[:, 0:256],
                    start=True, stop=True)

            def theta_round(hh):
                tp = pj.tile([128, 1024], F32, tag="pj", name=f"tp{hh}")
                for qc in range(2):
                    for kc in range(2):
                        nc.tensor.matmul(
                            tp[:, ts(qc, 512)],
                            wth_sb[:, kc, :],
                            xt_t[hh][:, kc, ts(qc, 512)],
                            start=(kc == 0),
                            stop=(kc == 1),
                        )
                cast_out(th_sb[:, ts(hh, 1024)], tp[:], bth_sb, None)

            # both theta rounds first (xt lands before the gpsimd xbf
            # transfer), then phi-hh0
            theta_round(0)
            # lazy DMA issues: wph for the phi rounds, wg for the g phase,
            # xb back half for phi hh2/3 + g tiles 16..31
            nc.scalar.dma_start(
                out=wph_sb[:], in_=_kc_pair_ap(d["wphT"], CI, 0, CI))
            nc.scalar.dma_start(
                out=wg_sb[:], in_=_kc_pair_ap(d["wgT"], CI, 0, CI))
            if bg_sb is not None:
                nc.scalar.dma_start(out=bg_sb[:], in_=d["bg"][:])
            nc.sync.dma_start(
                out=xbb_sb[:], in_=_kc_pair_ap(d["xb"], N, 2048, 2048))
            theta_round(1)
            pp = pj.tile([128, 1024], F32, tag="pj")
            for qc in range(2):
                for kc in range(2):
                    nc.tensor.matmul(
                        pp[:, ts(qc, 512)],
                        wph_sb[:, kc, :],
                        xbf_t[0][:, kc, ts(qc, 512)],
                        start=(kc == 0),
                        stop=(kc == 1),
                    )
            cast_out(ph_sb[:, 0:1024], pp[:], bph_sb, None)

        # ---- attention, software-pipelined against the remaining
        # projections: exp for s-tile i+8 is emitted behind the y-matmuls
        # of tile i, and the first 8 score/exp pairs precede the phi tail and
        # the whole g^T phase. phi-tail and g^T borrow the yps PSUM banks
        # (the y accumulation's start=True clears them afterwards).
        fF = [
            acts.tile([128, Q], BF16, tag=f"F{j}", name=f"F{j}")
            for j in range(4)
        ]
        ypsp_cm = tc.tile_pool(name="yps", bufs=1, space="PSUM")
        ypsp = ypsp_cm.__enter__()
        yps = ypsp.tile([128, Q], F32, tag="yps")
        scp_cm = tc.tile_pool(name="scp", bufs=2, space="PSUM")
        scp = scp_cm.__enter__()
        bp_cm = tc.tile_pool(name="bp", bufs=1)
        bp = bp_cm.__enter__()
        Bt = {}

        def sc_exp(i):
            B = bp.tile([128, Q], BF16, tag=f"B{i % N_BSLOT}", name=f"B{i}")
            Bt[i] = B
            for h in range(2):
                sc = scp.tile([128, 1024], F32, tag="sc")
                for qc in range(2):
                    nc.tensor.matmul(
                        sc[:, ts(qc, 512)],
                        ph_sb[:, ts(i, 128)],
                        th_sb[:, ts(h * 2 + qc, 512)],
                        start=True,
                        stop=True,
                    )
                if _dve_exp_half(i, h):
                    # Schraudolph exp on DVE: bf16 bits of exp(SCALE*sc)
                    nc.vector.tensor_scalar(
                        B[:, ts(h, 1024)].bitcast(I16), sc[:],
                        SCH_A, SCH_B, ALU.mult, ALU.add)
                else:
                    nc.scalar.activation(
                        B[:, ts(h, 1024)], sc[:], AF.Exp, scale=SCALE)
            # stride-4 fold chains, engine per FOLD_POOL LUT. The first add
            # of a chain sums B[j] and B[j+4] directly -- no seed copies.
            # Tile 31 is NOT folded into chain 3: it becomes the single
            # final add after the chain combines, so the tail critical path
            # past the last exp is one add. DVE folds for tiles 4..7 are
            # emitted later (after the g casts) so the g casts -- which gate
            # the whole y main loop -- aren't queued behind them.
            j = i % 4
            if i < 4 or i == 31:
                pass
            elif i in FOLD_POOL:
                if i == 4:
                    nc.gpsimd.tensor_tensor(fF[j][:], Bt[0][:], Bt[4][:], ALU.add)
                else:
                    nc.gpsimd.tensor_tensor(fF[j][:], fF[j][:], Bt[i][:], ALU.add)
            elif i >= 8:
                nc.vector.tensor_tensor(fF[j][:], fF[j][:], Bt[i][:], ALU.add)

        for i in range(8):
            sc_exp(i)

        # phi tail (tiles 8..31) into borrowed yps banks
        for hh in range(1, 4):
            pp = yps[:, ts(hh % 2, 1024)]
            if hh == 1:
                xbt, lc = xbf_t[1], 0
            else:
                xbt, lc = xbb_sb, (hh - 2) * 1024
            for qc in range(2):
                for kc in range(2):
                    nc.tensor.matmul(
                        pp[:, ts(qc, 512)],
                        wph_sb[:, kc, :],
                        xbt[:, kc, lc + qc * 512:lc + qc * 512 + 512],
                        start=(kc == 0),
                        stop=(kc == 1),
                    )
            cast_out(ph_sb[:, ts(hh, 1024)], pp[:], bph_sb, None)

        # g^T projection, also into borrowed yps banks. The casts split
        # between DVE and ACT: the y main loop can only begin once ALL g
        # casts have drained the borrowed banks, and DVE alone finishes
        # ~2us after the g matmuls do.
        for b in range(8):
            gp = yps[:, 512 * (b % 4):512 * (b % 4) + 512]
            for sj in range(4):
                st = b * 4 + sj
                xbt, lc = xb_at(st)
                for kc in range(2):
                    nc.tensor.matmul(
                        gp[:, ts(sj, 128)],
                        xbt[:, kc, lc:lc + 128],
                        wg_sb[:, kc, :],
                        start=(kc == 0),
                        stop=(kc == 1),
                    )
            if bg_sb is None and b % 2 == 1:
                nc.scalar.copy(gT_sb[:, ts(b, 512)], gp[:])
            else:
                cast_out(gT_sb[:, ts(b, 512)], gp[:], None, bg_sb)

        # deferred DVE first-adds for chains 1..3 (tiles 5,6,7), after the
        # g casts so those never wait behind exp-gated folds
        for i in (5, 6, 7):
            nc.vector.tensor_tensor(fF[i % 4][:], Bt[i % 4][:], Bt[i][:], ALU.add)

        # ---- main loop: y(i) first, then scores/exp for i+8, so the PE
        # queue head never blocks on an exp that is still in flight ----
        for i in range(32):
            B = Bt[i]
            for h in range(2):
                for qc in range(2):
                    nc.tensor.matmul(
                        yps[:, ts(h * 2 + qc, 512)],
                        gT_sb[:, ts(i, 128)],
                        B[:, ts(h * 2 + qc, 512)],
                        start=(i == 0),
                        stop=(i == 31),
                    )
            if i == 0:
                # tail-only weights, issued now so they never gate the
                # projections' DMA waits but still land long before use
                nc.sync.dma_start(out=wo_sb[:], in_=d["woT"][:])
                nc.sync.dma_start(out=id_sb[:], in_=d["ident"][:])
                if bo_sb is not None:
                    for oc in range(2):
                        nc.sync.dma_start(out=bo_sb[:, oc, :], in_=d["bo"][oc])
            if i < 24:
                sc_exp(i + 8)
        scp_cm.__exit__(None, None, None)

        # ---- tail, pipelined per 512-wide q-chunk:
        # d = ones @ {F0..F3, B31} (5-deep PSUM accumulation -- no DVE
        # combine chain at all) -> 1/d (approx) -> y*1/d -> {residual +
        # out-proj as one PSUM accumulation: ident@xt (start) + woT@ynt
        # (stop)} -> copy to SBUF -> DMA out.
        with (
            tc.tile_pool(name="dps", bufs=2, space="PSUM") as dpsp,
            tc.tile_pool(name="rps", bufs=2, space="PSUM") as rps,
        ):
            # dep-free dummy matmuls keep the PE busy while the last
            # exps/folds drain, so the HAM clock stays at 8/8 for the
            # tail's small matmuls (measured 2x difference)
            for _ in range(6):
                dwm = dpsp.tile([128, 512], F32, tag="dp")
                nc.tensor.matmul(
                    dwm[:, 0:256], ones_sb[:], wup_rhs[:, 0:256],
                    start=True, stop=True)
            for qc in range(4):
                dp = dpsp.tile([128, 512], F32, tag="dp")
                for fi in range(5):
                    src = fF[fi][:, ts(qc, 512)] if fi < 4 else (
                        Bt[31][:, ts(qc, 512)])
                    nc.tensor.matmul(
                        dp[:], ones_sb[:], src,
                        start=(fi == 0), stop=(fi == 4))
                rcp = outp.tile([128, 512], F32, tag="rcp")
                nc.vector.reciprocal_approx_fast(rcp[:], dp[:])
                ynt = outp.tile([128, 512], BF16, tag="ynt")
                nc.vector.tensor_tensor(
                    ynt[:], yps[:, ts(qc, 512)], rcp[:], ALU.mult)
                for oc in range(2):
                    rp = rps.tile([128, 512], F32, tag="rp")
                    xres = xt_t[qc // 2][:, oc, ts(qc % 2, 512)]
                    nc.tensor.matmul(
                        rp[:], id_sb[:], xres,
                        start=True, stop=False)
                    nc.tensor.matmul(
                        rp[:],
                        wo_sb[:, ts(oc, 128)],
                        ynt[:],
                        start=False,
                        stop=True,
                    )
                    # PSUM -> SBUF split across ACT (oc0) and DVE (oc1) so
                    # the copies run concurrently (DMA cannot read PSUM);
                    # out DMAs split across the sync and scalar queues
                    ot = outp.tile([128, 512], F32, tag=f"ot{oc}")
                    if bo_sb is not None:
                        nc.scalar.activation(
                            ot[:], rp[:], AF.Identity, bias=bo_sb[:, oc, :])
                    elif oc == 0:
                        nc.scalar.copy(ot[:], rp[:])
                    else:
                        nc.vector.tensor_copy(ot[:], rp[:])
                    [nc.sync, nc.scalar][oc].dma_start(
                        out=d["out"][oc][:, ts(qc, 512)], in_=ot[:])
        bp_cm.__exit__(None, None, None)
        ypsp_cm.__exit__(None, None, None)
        xin_cm.__exit__(None, None, None)


def _prep_in_maps(inputs):
    bf = ml_dtypes.bfloat16
    x = np.ascontiguousarray(np.asarray(inputs["x"], dtype=np.float32))
    w_g = np.asarray(inputs["w_g"], np.float32)
    b_g = np.asarray(inputs["b_g"], np.float32)
    w_theta = np.asarray(inputs["w_theta"], np.float32)
    b_theta = np.asarray(inputs["b_theta"], np.float32)
    w_phi = np.asarray(inputs["w_phi"], np.float32)
    b_phi = np.asarray(inputs["b_phi"], np.float32)
    w_out = np.asarray(inputs["w_out"], np.float32)
    b_out = np.asarray(inputs["b_out"], np.float32)

    flags = (
        bool(np.any(b_theta)), bool(np.any(b_phi)),
        bool(np.any(b_g)), bool(np.any(b_out)),
    )
    wthT = np.ascontiguousarray(w_theta.T).astype(bf).reshape(2, 128, CI)
    wphT = np.ascontiguousarray(w_phi.T).astype(bf).reshape(2, 128, CI)
    wgT = np.ascontiguousarray(w_g.T).astype(bf).reshape(2, 128, CI)
    woT = np.ascontiguousarray(w_out.T).astype(bf)          # [CI, C]
    ident = np.eye(128, dtype=bf)

    in_maps = []
    for c in range(NCORES):
        n, qh = c // 2, c % 2
        xr = x[n].reshape(C, N)
        xbc = xr.astype(bf)
        m = {
            "xb": np.ascontiguousarray(xbc.reshape(2, 128, N)),
            "xt": np.ascontiguousarray(
                xbc[:, qh * Q:(qh + 1) * Q].reshape(2, 128, Q)),
            "wthT": wthT, "wphT": wphT, "wgT": wgT, "woT": woT,
            "ident": ident,
        }
        if flags[0]:
            m["bth"] = np.ascontiguousarray(b_theta.reshape(128, 1))
        if flags[1]:
            m["bph"] = np.ascontiguousarray(b_phi.reshape(128, 1))
        if flags[2]:
            m["bg"] = np.ascontiguousarray(b_g.reshape(1, CI))
        if flags[3]:
            m["bo"] = np.ascontiguousarray(b_out.reshape(2, 128, 1))
        in_maps.append(m)
    return flags, in_maps


def _get_nc(flags):
    if flags not in _CACHE:
        _CACHE[flags] = _build(flags)
    return _CACHE[flags]


def kernel(**inputs):
    flags, in_maps = _prep_in_maps(inputs)
    nc = _get_nc(flags)
    res = run_bass_kernel_spmd(nc, in_maps, list(range(NCORES)))
    out = np.empty((NB, C, N), np.float32)
    for c in range(NCORES):
        n, qh = c // 2, c % 2
        out[n][:, qh * Q:(qh + 1) * Q] = res.results[c]["out"].reshape(C, Q)
    return out.reshape(NB, C, 64, 64)


if __name__ == "__main__":
    x = np.random.randn(NB, C, 64, 64).astype(np.float32) * 0.1
    rng = np.random.default_rng(0)
    ins = {
        "x": x,
        "w_g": rng.normal(size=(CI, C)).astype(np.float32) * 0.01,
        "b_g": np.zeros(CI, np.float32),
        "w_theta": rng.normal(size=(CI, C)).astype(np.float32) * 0.01,
        "b_theta": np.zeros(CI, np.float32),
        "w_phi": rng.normal(size=(CI, C)).astype(np.float32) * 0.01,
        "b_phi": np.zeros(CI, np.float32),
        "w_out": rng.normal(size=(C, CI)).astype(np.float32) * 0.01,
        "b_out": np.zeros(C, np.float32),
    }
    o = kernel(**ins)
    print("ok", o.shape, o.dtype)


# revision 43
# speedup vs baseline: 1.2667x; 1.0401x over previous
"""NonLocal2D (attention) block on 8 trn2 NeuronCores.

Sharding: core c -> batch n = c//2, query-half qh = c%2 (2048 of the 4096
spatial positions). Each core receives the full x[n] (so phi/g are computed
locally -- no collectives) plus its own query slice, and produces
out[n][:, qh*2048:(qh+1)*2048].

Per-core dataflow (layouts chosen so no transposes are ever needed):
  theta:    [CI=128, Q]  = wthT-chunks (lhsT) @ xt-chunks (rhs)     [PE]
  phi:      [CI=128, N]  = wphT-chunks (lhsT) @ xb-chunks (rhs)     [PE]
  g^T:      [s, CI] tiles = xb-chunks (lhsT) @ wgT-chunks (rhs)     [PE]
  scores^T: [s=128, q=1024] = phi-tile (lhsT) @ theta (rhs)         [PE -> PSUM f32]
  B = exp(SCALE*scores^T) -> bf16 SBUF; most half-tiles on ACT, a
      subset on DVE via the Schraudolph int16-bits trick (bf16 bits =
      trunc(x*128/ln2 + 127*128 - c))  (no max-sub: |scaled| < ~30)  [ACT+DVE]
  denom: 4 stride-4 fold chains over B tiles (split GPSIMD/DVE via a
      static LUT; the first add of each chain combines two B tiles so
      no copies are needed), combined to one F, then
      d = ones (lhsT) @ F per q-chunk                               [DVE/GPSIMD/PE]
  y^T += gT-tile (lhsT) @ B   (PSUM accumulate over 32 s-tiles)     [PE]
  y_norm^T = y^T * (1/d) -> bf16                                    [DVE]
  out-proj + residual in one PSUM group:
      rp = ident (lhsT) @ xt-chunk  (start)  -- the +x residual
      rp += woT-chunk (lhsT) @ y_norm^T (stop); DMA out from PSUM   [PE]

The residual uses the bf16 xt copy (no separate f32 x load): the extra
~2^-9 relative rounding on x costs ~1.7e-3 rel err, far under the 2e-2
gate, and saves 4MB/core of input DMA.
"""

import numpy as np
import ml_dtypes

import concourse.bass as bass
import concourse.mybir as mybir
import concourse.tile as tile
from concourse import bacc
from concourse.bass import ts
from concourse.bass_utils import run_bass_kernel_spmd

BF16 = mybir.dt.bfloat16
F32 = mybir.dt.float32
I16 = mybir.dt.int16
AF = mybir.ActivationFunctionType
ALU = mybir.AluOpType

C = 256          # in channels
CI = 128         # inter channels
NB = 4           # batch
N = 4096         # H*W
Q = 2048         # queries per core
NCORES = 8
SCALE = float(128 ** 0.5)   # reference divides by d**-0.5

# Schraudolph exp in bf16-bit space: bits = trunc(x*SCALE*128/ln2 + b)
SCH_A = SCALE * 128.0 / float(np.log(2.0))
SCH_B = 127.0 * 128.0 - 5.0

N_BSLOT = 24     # B-tile ring (WAR distance 24 >> pipeline depth)

# fold engine LUT: which tiles' fold-adds run on GPSIMD (rest on DVE).
# Exactly chain 0 (j = i%4 == 0): a whole chain per engine, so no
# cross-engine handoff ever head-blocks the DVE queue.
FOLD_POOL = {4, 8, 12, 16, 20, 24, 28}

_CACHE: dict = {}


def _dve_exp_half(i, h):
    # which exp half-tiles run on DVE (Schraudolph) instead of ACT.
    # Tiles 30/31 stay fully on ACT (the last exps gate the tail and DVE
    # still owes their folds); both halves of 29 go to DVE so ACT
    # reaches the final exps sooner.
    return (h == 0 and i % 3 == 1 and i < 29) or i == 29


def _build(flags):
    bth_nz, bph_nz, bg_nz, bo_nz = flags
    nc = bacc.Bacc("TRN2", target_bir_lowering=False, debug=False)

    d = {}
    d["xb"] = nc.dram_tensor("xb", [2, 128, N], BF16, kind="ExternalInput").ap()
    d["xt"] = nc.dram_tensor("xt", [2, 128, Q], BF16, kind="ExternalInput").ap()
    d["wthT"] = nc.dram_tensor("wthT", [2, 128, CI], BF16, kind="ExternalInput").ap()
    d["wphT"] = nc.dram_tensor("wphT", [2, 128, CI], BF16, kind="ExternalInput").ap()
    d["wgT"] = nc.dram_tensor("wgT", [2, 128, CI], BF16, kind="ExternalInput").ap()
    d["woT"] = nc.dram_tensor("woT", [128, C], BF16, kind="ExternalInput").ap()
    d["ident"] = nc.dram_tensor("ident", [128, 128], BF16, kind="ExternalInput").ap()
    d["bth"] = nc.dram_tensor("bth", [128, 1], F32, kind="ExternalInput").ap() if bth_nz else None
    d["bph"] = nc.dram_tensor("bph", [128, 1], F32, kind="ExternalInput").ap() if bph_nz else None
    d["bg"] = nc.dram_tensor("bg", [1, CI], F32, kind="ExternalInput").ap() if bg_nz else None
    d["bo"] = nc.dram_tensor("bo", [2, 128, 1], F32, kind="ExternalInput").ap() if bo_nz else None
    d["out"] = nc.dram_tensor("out", [2, 128, Q], F32, kind="ExternalOutput").ap()

    with tile.TileContext(nc) as tc:
        _bass_body(tc, d)
    nc.compile()
    return nc


def _kc_pair_ap(dram_ap, cols, col0, count):
    """3D dram AP reading [2,128,cols] as [p=128, kc=2, count] at col0."""
    return bass.AP(
        tensor=dram_ap.tensor,
        offset=col0,
        ap=[[cols, 128], [128 * cols, 2], [1, count]],
    )


def _bass_body(tc, d):
    nc = tc.nc

    with (
        tc.tile_pool(name="const", bufs=1) as const,
        tc.tile_pool(name="acts", bufs=1) as acts,
        tc.tile_pool(name="outs", bufs=2) as outp,
    ):
        # ---- constants / weights ----
        # memsets on DVE (idle at t=0) so the PE warm-up never waits on the
        # gpsimd program load; the exp-table warm reads scratch itself
        # (garbage in, table warmed) so it carries no cross-engine dep
        ones_sb = const.tile([128, 128], BF16, tag="ones")
        nc.vector.memset(ones_sb[:], 1.0)
        wup_rhs = const.tile([128, 512], BF16, tag="wup_rhs")
        nc.vector.memset(wup_rhs[:], 0.0)
        scratch = const.tile([128, 1], BF16, tag="scratch")
        nc.vector.memset(scratch[:], 1.0)

        wth_sb = const.tile([128, 2, CI], BF16, tag="wth")
        wph_sb = const.tile([128, 2, CI], BF16, tag="wph")
        wg_sb = const.tile([128, 2, CI], BF16, tag="wg")
        wo_sb = const.tile([128, C], BF16, tag="wo")
        id_sb = const.tile([128, 128], BF16, tag="ident")
        bth_sb = bph_sb = bg_sb = bo_sb = None
        if d["bth"] is not None:
            bth_sb = const.tile([128, 1], F32, tag="bth")
        if d["bph"] is not None:
            bph_sb = const.tile([128, 1], F32, tag="bph")
        if d["bg"] is not None:
            bg_sb = const.tile([1, CI], F32, tag="bg")
        if d["bo"] is not None:
            bo_sb = const.tile([128, 2, 1], F32, tag="bo")

        th_sb = acts.tile([128, Q], BF16, tag="th")
        ph_sb = acts.tile([128, N], BF16, tag="ph")
        gT_sb = acts.tile([128, 32 * CI], BF16, tag="gT")  # tile i at cols [128i, 128i+128)

        # ---- input fill: one tile per DMA transfer so every consumer's
        # RAW dep is exactly one transfer (no waiting on sibling chunks),
        # ordered by first use across three queues ----
        xin_cm = tc.tile_pool(name="xin", bufs=1)
        xin = xin_cm.__enter__()
        xt_t = [
            xin.tile([128, 2, 1024], BF16, tag=f"xt{hh}", name=f"xt{hh}")
            for hh in range(2)
        ]
        xbf_t = [
            xin.tile([128, 2, 1024], BF16, tag=f"xbf{c}", name=f"xbf{c}")
            for c in range(2)
        ]
        xbb_sb = xin.tile([128, 2, 2048], BF16, tag="xbb")

        # Only the transfers needed FIRST are issued here. DMA waits appear
        # to coalesce to "every transfer issued so far on that queue", so
        # later-needed tensors (wg, wo, ident, xb back half, biases) are
        # issued lazily at their point of need to keep theta's wait minimal.
        # sync: xt halves (theta); scalar: wth; gpsimd: front half of xb
        for hh in range(2):
            nc.sync.dma_start(
                out=xt_t[hh][:],
                in_=_kc_pair_ap(d["xt"], Q, hh * 1024, 1024))
        nc.scalar.dma_start(out=wth_sb[:], in_=_kc_pair_ap(d["wthT"], CI, 0, CI))
        if bth_sb is not None:
            nc.scalar.dma_start(out=bth_sb[:], in_=d["bth"][:])
        if bph_sb is not None:
            nc.scalar.dma_start(out=bph_sb[:], in_=d["bph"][:])
        # xbf front quarter on sync (hwdge, lands right after xt so phi-hh0
        # isn't gated on the slow gpsimd swdge path); second quarter swdge
        nc.sync.dma_start(
            out=xbf_t[0][:], in_=_kc_pair_ap(d["xb"], N, 0, 1024))
        nc.gpsimd.dma_start(
            out=xbf_t[1][:], in_=_kc_pair_ap(d["xb"], N, 1024, 1024))

        def xb_at(st128):
            # (tile, local col0) for xb column st128*128
            col = st128 * 128
            if col < 1024:
                return xbf_t[0], col
            if col < 2048:
                return xbf_t[1], col - 1024
            return xbb_sb, col - 2048

        # warm the exp table set early so the first real exp isn't +2.7us
        nc.scalar.activation(scratch[:], scratch[:], AF.Exp, scale=1.0)


        def cast_out(dst_ap, src_psum, bias_part, bias_row):
            # PSUM f32 -> SBUF bf16, optionally + bias
            if bias_part is not None:
                nc.vector.tensor_scalar_add(dst_ap, src_psum, bias_part[:])
            elif bias_row is not None:
                bcast = bass.AP(
                    tensor=bias_row.tensor,
                    offset=bias_row.offset,
                    ap=[[0, 128], [0, 4], [1, CI]],
                )
                nc.vector.tensor_tensor(dst_ap, src_psum, bcast, ALU.add)
            else:
                nc.vector.tensor_copy(dst_ap, src_psum)

        # ---- theta + first quarter of phi (enough for 8 s-tiles) ----
        with (
            tc.tile_pool(name="pj", bufs=2, space="PSUM") as pj,
            tc.tile_pool(name="wup", bufs=1, space="PSUM") as wup,
        ):
            # PE warm-up during the DMA fill: dummy matmuls flip the HAM
            # clock gate toward 8/8 before the first real matmul issues.
            # 8 of them (~4us at the cold clock) end right as the first
            # xt/wth transfers land -- more would push theta out.
            wps = wup.tile([128, 512], F32, tag="wps")
            for _ in range(8):
                nc.tensor.matmul(
                    wps[:, 0:256], ones_sb[:], wup_rhs[:, 0:256],
                    start=True, stop=True)

            def theta_round(hh):
                tp = pj.tile([128, 1024], F32, tag="pj", name=f"tp{hh}")
                for qc in range(2):
                    for kc in range(2):
                        nc.tensor.matmul(
                            tp[:, ts(qc, 512)],
                            wth_sb[:, kc, :],
                            xt_t[hh][:, kc, ts(qc, 512)],
                            start=(kc == 0),
                            stop=(kc == 1),
                        )
                cast_out(th_sb[:, ts(hh, 1024)], tp[:], bth_sb, None)

            # both theta rounds first (xt lands before the gpsimd xbf
            # transfer), then phi-hh0
            theta_round(0)
            # lazy DMA issues: wph for the phi rounds, wg for the g phase,
            # xb back half for phi hh2/3 + g tiles 16..31
            nc.scalar.dma_start(
                out=wph_sb[:], in_=_kc_pair_ap(d["wphT"], CI, 0, CI))
            nc.scalar.dma_start(
                out=wg_sb[:], in_=_kc_pair_ap(d["wgT"], CI, 0, CI))
            if bg_sb is not None:
                nc.scalar.dma_start(out=bg_sb[:], in_=d["bg"][:])
            nc.sync.dma_start(
                out=xbb_sb[:], in_=_kc_pair_ap(d["xb"], N, 2048, 2048))
            theta_round(1)
            pp = pj.tile([128, 1024], F32, tag="pj")
            for qc in range(2):
                for kc in range(2):
                    nc.tensor.matmul(
                        pp[:, ts(qc, 512)],
                        wph_sb[:, kc, :],
                        xbf_t[0][:, kc, ts(qc, 512)],
                        start=(kc == 0),
                        stop=(kc == 1),
                    )
            cast_out(ph_sb[:, 0:1024], pp[:], bph_sb, None)

        # ---- attention, software-pipelined against the remaining
        # projections: exp for s-tile i+8 is emitted behind the y-matmuls
        # of tile i, and the first 8 score/exp pairs precede the phi tail and
        # the whole g^T phase. phi-tail and g^T borrow the yps PSUM banks
        # (the y accumulation's start=True clears them afterwards).
        fF = [
            acts.tile([128, Q], BF16, tag=f"F{j}", name=f"F{j}")
            for j in range(4)
        ]
        # yps as four independent [128,512] chunk tiles: the y main loop's
        # chunk-qc matmul then only WAR-waits the g casts that borrowed
        # THAT chunk (a single [128,2048] tile made y(0) wait all eight)
        ypsp_cm = tc.tile_pool(name="yps", bufs=1, space="PSUM")
        ypsp = ypsp_cm.__enter__()
        yps_t = [
            ypsp.tile([128, 512], F32, tag=f"yps{c}", name=f"yps{c}")
            for c in range(4)
        ]
        scp_cm = tc.tile_pool(name="scp", bufs=2, space="PSUM")
        scp = scp_cm.__enter__()
        bp_cm = tc.tile_pool(name="bp", bufs=1)
        bp = bp_cm.__enter__()
        Bt = {}

        def sc_exp(i):
            B = bp.tile([128, Q], BF16, tag=f"B{i % N_BSLOT}", name=f"B{i}")
            Bt[i] = B
            for h in range(2):
                sc = scp.tile([128, 1024], F32, tag="sc")
                for qc in range(2):
                    nc.tensor.matmul(
                        sc[:, ts(qc, 512)],
                        ph_sb[:, ts(i, 128)],
                        th_sb[:, ts(h * 2 + qc, 512)],
                        start=True,
                        stop=True,
                    )
                if _dve_exp_half(i, h):
                    # Schraudolph exp on DVE: bf16 bits of exp(SCALE*sc)
                    nc.vector.tensor_scalar(
                        B[:, ts(h, 1024)].bitcast(I16), sc[:],
                        SCH_A, SCH_B, ALU.mult, ALU.add)
                else:
                    nc.scalar.activation(
                        B[:, ts(h, 1024)], sc[:], AF.Exp, scale=SCALE)
            # stride-4 fold chains, engine per FOLD_POOL LUT. The first add
            # of a chain sums B[j] and B[j+4] directly -- no seed copies.
            # Tile 31 is NOT folded into chain 3: it becomes the single
            # final add after the chain combines, so the tail critical path
            # past the last exp is one add. DVE folds for tiles 4..7 are
            # emitted later (after the g casts) so the g casts -- which gate
            # the whole y main loop -- aren't queued behind them.
            j = i % 4
            if i < 4 or i == 31:
                pass
            elif i in FOLD_POOL:
                if i == 4:
                    nc.gpsimd.tensor_tensor(fF[j][:], Bt[0][:], Bt[4][:], ALU.add)
                else:
                    nc.gpsimd.tensor_tensor(fF[j][:], fF[j][:], Bt[i][:], ALU.add)
            elif i >= 8:
                nc.vector.tensor_tensor(fF[j][:], fF[j][:], Bt[i][:], ALU.add)

        for i in range(8):
            sc_exp(i)

        # phi tail (tiles 8..31) into borrowed yps chunk tiles
        for hh in range(1, 4):
            c0 = (hh % 2) * 2
            if hh == 1:
                xbt, lc = xbf_t[1], 0
            else:
                xbt, lc = xbb_sb, (hh - 2) * 1024
            for qc in range(2):
                for kc in range(2):
                    nc.tensor.matmul(
                        yps_t[c0 + qc][:],
                        wph_sb[:, kc, :],
                        xbt[:, kc, lc + qc * 512:lc + qc * 512 + 512],
                        start=(kc == 0),
                        stop=(kc == 1),
                    )
            for qc in range(2):
                cast_out(
                    ph_sb[:, hh * 1024 + qc * 512:hh * 1024 + qc * 512 + 512],
                    yps_t[c0 + qc][:], bph_sb, None)

        # g^T projection, also into borrowed yps banks. The casts split
        # between DVE and ACT: the y main loop can only begin once ALL g
        # casts have drained the borrowed banks, and DVE alone finishes
        # ~2us after the g matmuls do.
        for b in range(8):
            gp = yps_t[b % 4][:]
            for sj in range(4):
                st = b * 4 + sj
                xbt, lc = xb_at(st)
                for kc in range(2):
                    nc.tensor.matmul(
                        gp[:, ts(sj, 128)],
                        xbt[:, kc, lc:lc + 128],
                        wg_sb[:, kc, :],
                        start=(kc == 0),
                        stop=(kc == 1),
                    )
            if bg_sb is None and b % 2 == 1:
                nc.scalar.copy(gT_sb[:, ts(b, 512)], gp[:])
            else:
                cast_out(gT_sb[:, ts(b, 512)], gp[:], None, bg_sb)

        # deferred DVE first-adds for chains 1..3 (tiles 5,6,7), after the
        # g casts so those never wait behind exp-gated folds
        for i in (5, 6, 7):
            nc.vector.tensor_tensor(fF[i % 4][:], Bt[i % 4][:], Bt[i][:], ALU.add)

        # ---- main loop: y(i) first, then scores/exp for i+8, so the PE
        # queue head never blocks on an exp that is still in flight ----
        for i in range(32):
            B = Bt[i]
            for h in range(2):
                for qc in range(2):
                    nc.tensor.matmul(
                        yps_t[h * 2 + qc][:],
                        gT_sb[:, ts(i, 128)],
                        B[:, ts(h * 2 + qc, 512)],
                        start=(i == 0),
                        stop=(i == 31),
                    )
            if i == 0:
                # tail-only weights, issued now so they never gate the
                # projections' DMA waits but still land long before use
                nc.sync.dma_start(out=wo_sb[:], in_=d["woT"][:])
                nc.sync.dma_start(out=id_sb[:], in_=d["ident"][:])
                if bo_sb is not None:
                    for oc in range(2):
                        nc.sync.dma_start(out=bo_sb[:, oc, :], in_=d["bo"][oc])
            if i < 24:
                sc_exp(i + 8)
        scp_cm.__exit__(None, None, None)

        # ---- tail, pipelined per 512-wide q-chunk:
        # d = ones @ {F0..F3, B31} (5-deep PSUM accumulation -- no DVE
        # combine chain at all) -> 1/d (approx) -> y*1/d -> {residual +
        # out-proj as one PSUM accumulation: ident@xt (start) + woT@ynt
        # (stop)} -> copy to SBUF -> DMA out.
        with (
            tc.tile_pool(name="dps", bufs=2, space="PSUM") as dpsp,
            tc.tile_pool(name="rps", bufs=2, space="PSUM") as rps,
        ):
            # dep-free dummy matmuls keep the PE busy while the last
            # exps/folds drain, so the HAM clock stays at 8/8 for the
            # tail's small matmuls (measured 2x difference)
            for _ in range(6):
                dwm = dpsp.tile([128, 512], F32, tag="dp")
                nc.tensor.matmul(
                    dwm[:, 0:256], ones_sb[:], wup_rhs[:, 0:256],
                    start=True, stop=True)
            for qc in range(4):
                dp = dpsp.tile([128, 512], F32, tag="dp")
                for fi in range(5):
                    src = fF[fi][:, ts(qc, 512)] if fi < 4 else (
                        Bt[31][:, ts(qc, 512)])
                    nc.tensor.matmul(
                        dp[:], ones_sb[:], src,
                        start=(fi == 0), stop=(fi == 4))
                rcp = outp.tile([128, 512], F32, tag="rcp")
                nc.vector.reciprocal_approx_fast(rcp[:], dp[:])
                ynt = outp.tile([128, 512], BF16, tag="ynt")
                nc.vector.tensor_tensor(
                    ynt[:], yps_t[qc][:], rcp[:], ALU.mult)
                for oc in range(2):
                    rp = rps.tile([128, 512], F32, tag="rp")
                    xres = xt_t[qc // 2][:, oc, ts(qc % 2, 512)]
                    nc.tensor.matmul(
                        rp[:], id_sb[:], xres,
                        start=True, stop=False)
                    nc.tensor.matmul(
                        rp[:],
                        wo_sb[:, ts(oc, 128)],
                        ynt[:],
                        start=False,
                        stop=True,
                    )
                    # PSUM -> SBUF split across ACT (oc0) and DVE (oc1) so
                    # the copies run concurrently (DMA cannot read PSUM);
                    # out DMAs split across the sync and scalar queues
                    ot = outp.tile([128, 512], F32, tag=f"ot{oc}")
                    if bo_sb is not None:
                        nc.scalar.activation(
                            ot[:], rp[:], AF.Identity, bias=bo_sb[:, oc, :])
                    elif oc == 0:
                        nc.scalar.copy(ot[:], rp[:])
                    else:
                        nc.vector.tensor_copy(ot[:], rp[:])
                    [nc.sync, nc.scalar][oc].dma_start(
                        out=d["out"][oc][:, ts(qc, 512)], in_=ot[:])
        bp_cm.__exit__(None, None, None)
        ypsp_cm.__exit__(None, None, None)
        xin_cm.__exit__(None, None, None)


def _prep_in_maps(inputs):
    bf = ml_dtypes.bfloat16
    x = np.ascontiguousarray(np.asarray(inputs["x"], dtype=np.float32))
    w_g = np.asarray(inputs["w_g"], np.float32)
    b_g = np.asarray(inputs["b_g"], np.float32)
    w_theta = np.asarray(inputs["w_theta"], np.float32)
    b_theta = np.asarray(inputs["b_theta"], np.float32)
    w_phi = np.asarray(inputs["w_phi"], np.float32)
    b_phi = np.asarray(inputs["b_phi"], np.float32)
    w_out = np.asarray(inputs["w_out"], np.float32)
    b_out = np.asarray(inputs["b_out"], np.float32)

    flags = (
        bool(np.any(b_theta)), bool(np.any(b_phi)),
        bool(np.any(b_g)), bool(np.any(b_out)),
    )
    wthT = np.ascontiguousarray(w_theta.T).astype(bf).reshape(2, 128, CI)
    wphT = np.ascontiguousarray(w_phi.T).astype(bf).reshape(2, 128, CI)
    wgT = np.ascontiguousarray(w_g.T).astype(bf).reshape(2, 128, CI)
    woT = np.ascontiguousarray(w_out.T).astype(bf)          # [CI, C]
    ident = np.eye(128, dtype=bf)

    in_maps = []
    for c in range(NCORES):
        n, qh = c // 2, c % 2
        xr = x[n].reshape(C, N)
        xbc = xr.astype(bf)
        m = {
            "xb": np.ascontiguousarray(xbc.reshape(2, 128, N)),
            "xt": np.ascontiguousarray(
                xbc[:, qh * Q:(qh + 1) * Q].reshape(2, 128, Q)),
            "wthT": wthT, "wphT": wphT, "wgT": wgT, "woT": woT,
            "ident": ident,
        }
        if flags[0]:
            m["bth"] = np.ascontiguousarray(b_theta.reshape(128, 1))
        if flags[1]:
            m["bph"] = np.ascontiguousarray(b_phi.reshape(128, 1))
        if flags[2]:
            m["bg"] = np.ascontiguousarray(b_g.reshape(1, CI))
        if flags[3]:
            m["bo"] = np.ascontiguousarray(b_out.reshape(2, 128, 1))
        in_maps.append(m)
    return flags, in_maps


def _get_nc(flags):
    if flags not in _CACHE:
        _CACHE[flags] = _build(flags)
    return _CACHE[flags]


def kernel(**inputs):
    flags, in_maps = _prep_in_maps(inputs)
    nc = _get_nc(flags)
    res = run_bass_kernel_spmd(nc, in_maps, list(range(NCORES)))
    out = np.empty((NB, C, N), np.float32)
    for c in range(NCORES):
        n, qh = c // 2, c % 2
        out[n][:, qh * Q:(qh + 1) * Q] = res.results[c]["out"].reshape(C, Q)
    return out.reshape(NB, C, 64, 64)


if __name__ == "__main__":
    x = np.random.randn(NB, C, 64, 64).astype(np.float32) * 0.1
    rng = np.random.default_rng(0)
    ins = {
        "x": x,
        "w_g": rng.normal(size=(CI, C)).astype(np.float32) * 0.01,
        "b_g": np.zeros(CI, np.float32),
        "w_theta": rng.normal(size=(CI, C)).astype(np.float32) * 0.01,
        "b_theta": np.zeros(CI, np.float32),
        "w_phi": rng.normal(size=(CI, C)).astype(np.float32) * 0.01,
        "b_phi": np.zeros(CI, np.float32),
        "w_out": rng.normal(size=(C, CI)).astype(np.float32) * 0.01,
        "b_out": np.zeros(C, np.float32),
    }
    o = kernel(**ins)
    print("ok", o.shape, o.dtype)


# revision 44
# speedup vs baseline: 1.3066x; 1.0315x over previous
"""NonLocal2D (attention) block on 8 trn2 NeuronCores.

Sharding: core c -> batch n = c//2, query-half qh = c%2 (2048 of the 4096
spatial positions). Each core receives the full x[n] (so phi/g are computed
locally -- no collectives) plus its own query slice, and produces
out[n][:, qh*2048:(qh+1)*2048].

Per-core dataflow (layouts chosen so no transposes are ever needed):
  theta:    [CI=128, Q]  = wthT-chunks (lhsT) @ xt-chunks (rhs)     [PE]
  phi:      [CI=128, N]  = wphT-chunks (lhsT) @ xb-chunks (rhs)     [PE]
  g^T:      [s, CI] tiles = xb-chunks (lhsT) @ wgT-chunks (rhs)     [PE]
  scores^T: [s=128, q=1024] = phi-tile (lhsT) @ theta (rhs)         [PE -> PSUM f32]
  B = exp(SCALE*scores^T) -> bf16 SBUF; most half-tiles on ACT, a
      subset on DVE via the Schraudolph int16-bits trick (bf16 bits =
      trunc(x*128/ln2 + 127*128 - c))  (no max-sub: |scaled| < ~30)  [ACT+DVE]
  denom: 4 stride-4 fold chains over B tiles (split GPSIMD/DVE via a
      static LUT; the first add of each chain combines two B tiles so
      no copies are needed), combined to one F, then
      d = ones (lhsT) @ F per q-chunk                               [DVE/GPSIMD/PE]
  y^T += gT-tile (lhsT) @ B   (PSUM accumulate over 32 s-tiles)     [PE]
  y_norm^T = y^T * (1/d) -> bf16                                    [DVE]
  out-proj + residual in one PSUM group:
      rp = ident (lhsT) @ xt-chunk  (start)  -- the +x residual
      rp += woT-chunk (lhsT) @ y_norm^T (stop); DMA out from PSUM   [PE]

The residual uses the bf16 xt copy (no separate f32 x load): the extra
~2^-9 relative rounding on x costs ~1.7e-3 rel err, far under the 2e-2
gate, and saves 4MB/core of input DMA.
"""

import numpy as np
import ml_dtypes

import concourse.bass as bass
import concourse.mybir as mybir
import concourse.tile as tile
from concourse import bacc
from concourse.bass import ts
from concourse.bass_utils import run_bass_kernel_spmd

BF16 = mybir.dt.bfloat16
F32 = mybir.dt.float32
F8 = mybir.dt.float8e4
I16 = mybir.dt.int16
DR = mybir.MatmulPerfMode.DoubleRow
AF = mybir.ActivationFunctionType
ALU = mybir.AluOpType

C = 256          # in channels
CI = 128         # inter channels
NB = 4           # batch
N = 4096         # H*W
Q = 2048         # queries per core
NCORES = 8
SCALE = float(128 ** 0.5)   # reference divides by d**-0.5

# Schraudolph exp in bf16-bit space: bits = trunc(x*SCALE*128/ln2 + b)
SCH_A = SCALE * 128.0 / float(np.log(2.0))
SCH_B = 127.0 * 128.0 - 5.0

N_BSLOT = 24     # B-tile ring (WAR distance 24 >> pipeline depth)

# fold engine LUT: which tiles' fold-adds run on GPSIMD (rest on DVE).
# Exactly chain 0 (j = i%4 == 0): a whole chain per engine, so no
# cross-engine handoff ever head-blocks the DVE queue.
FOLD_POOL = {4, 8, 12, 16, 20, 24, 28}

_CACHE: dict = {}


def _dve_exp_half(i, h):
    # which exp half-tiles run on DVE (Schraudolph) instead of ACT.
    # Tiles 30/31 stay fully on ACT (the last exps gate the tail and DVE
    # still owes their folds); both halves of 29 go to DVE so ACT
    # reaches the final exps sooner.
    return (h == 0 and i % 3 == 1 and i < 29) or i == 29


def _build(flags):
    bth_nz, bph_nz, bg_nz, bo_nz = flags
    nc = bacc.Bacc("TRN2", target_bir_lowering=False, debug=False)

    d = {}
    d["xb"] = nc.dram_tensor("xb", [2, 128, N], F8, kind="ExternalInput").ap()
    d["xt"] = nc.dram_tensor("xt", [2, 128, Q], BF16, kind="ExternalInput").ap()
    d["xtf8"] = nc.dram_tensor("xtf8", [2, 128, Q], F8, kind="ExternalInput").ap()
    d["wthT"] = nc.dram_tensor("wthT", [2, 128, CI], F8, kind="ExternalInput").ap()
    d["wphT"] = nc.dram_tensor("wphT", [2, 128, CI], F8, kind="ExternalInput").ap()
    d["wgT"] = nc.dram_tensor("wgT", [2, 128, CI], F8, kind="ExternalInput").ap()
    d["woT"] = nc.dram_tensor("woT", [128, C], BF16, kind="ExternalInput").ap()
    d["ident"] = nc.dram_tensor("ident", [128, 128], BF16, kind="ExternalInput").ap()
    d["bth"] = nc.dram_tensor("bth", [128, 1], F32, kind="ExternalInput").ap() if bth_nz else None
    d["bph"] = nc.dram_tensor("bph", [128, 1], F32, kind="ExternalInput").ap() if bph_nz else None
    d["bg"] = nc.dram_tensor("bg", [1, CI], F32, kind="ExternalInput").ap() if bg_nz else None
    d["bo"] = nc.dram_tensor("bo", [2, 128, 1], F32, kind="ExternalInput").ap() if bo_nz else None
    d["out"] = nc.dram_tensor("out", [2, 128, Q], F32, kind="ExternalOutput").ap()

    with tile.TileContext(nc) as tc:
        _bass_body(tc, d)
    nc.compile()
    return nc


def _kc_pair_ap(dram_ap, cols, col0, count):
    """3D dram AP reading [2,128,cols] as [p=128, kc=2, count] at col0."""
    return bass.AP(
        tensor=dram_ap.tensor,
        offset=col0,
        ap=[[cols, 128], [128 * cols, 2], [1, count]],
    )


def _bass_body(tc, d):
    nc = tc.nc

    with (
        tc.tile_pool(name="const", bufs=1) as const,
        tc.tile_pool(name="acts", bufs=1) as acts,
        tc.tile_pool(name="outs", bufs=2) as outp,
    ):
        # ---- constants / weights ----
        # memsets on DVE (idle at t=0) so the PE warm-up never waits on the
        # gpsimd program load; the exp-table warm reads scratch itself
        # (garbage in, table warmed) so it carries no cross-engine dep
        ones_sb = const.tile([128, 128], BF16, tag="ones")
        nc.vector.memset(ones_sb[:], 1.0)
        wup_rhs = const.tile([128, 512], BF16, tag="wup_rhs")
        nc.vector.memset(wup_rhs[:], 0.0)
        scratch = const.tile([128, 1], BF16, tag="scratch")
        nc.vector.memset(scratch[:], 1.0)

        wth_sb = const.tile([128, 2, CI], F8, tag="wth")
        wph_sb = const.tile([128, 2, CI], F8, tag="wph")
        wg_sb = const.tile([128, 2, CI], F8, tag="wg")
        wo_sb = const.tile([128, C], BF16, tag="wo")
        id_sb = const.tile([128, 128], BF16, tag="ident")
        bth_sb = bph_sb = bg_sb = bo_sb = None
        if d["bth"] is not None:
            bth_sb = const.tile([128, 1], F32, tag="bth")
        if d["bph"] is not None:
            bph_sb = const.tile([128, 1], F32, tag="bph")
        if d["bg"] is not None:
            bg_sb = const.tile([1, CI], F32, tag="bg")
        if d["bo"] is not None:
            bo_sb = const.tile([128, 2, 1], F32, tag="bo")

        th_sb = acts.tile([128, Q], BF16, tag="th")
        ph_sb = acts.tile([128, N], BF16, tag="ph")
        gT_sb = acts.tile([128, 32 * CI], BF16, tag="gT")  # tile i at cols [128i, 128i+128)

        # ---- input fill: one tile per DMA transfer so every consumer's
        # RAW dep is exactly one transfer (no waiting on sibling chunks),
        # ordered by first use across three queues ----
        xin_cm = tc.tile_pool(name="xin", bufs=1)
        xin = xin_cm.__enter__()
        xt_t = [
            xin.tile([128, 2, 1024], BF16, tag=f"xt{hh}", name=f"xt{hh}")
            for hh in range(2)
        ]
        xt8_t = [
            xin.tile([128, 2, 1024], F8, tag=f"xt8{hh}", name=f"xt8{hh}")
            for hh in range(2)
        ]
        xbf_t = [
            xin.tile([128, 2, 1024], F8, tag=f"xbf{c}", name=f"xbf{c}")
            for c in range(2)
        ]
        xbb_sb = xin.tile([128, 2, 2048], F8, tag="xbb")

        # Only the transfers needed FIRST are issued here. DMA waits appear
        # to coalesce to "every transfer issued so far on that queue", so
        # later-needed tensors (wg, wo, ident, xb back half, biases) are
        # issued lazily at their point of need to keep theta's wait minimal.
        # sync: xt halves (theta); scalar: wth; gpsimd: front half of xb
        for hh in range(2):
            nc.sync.dma_start(
                out=xt8_t[hh][:],
                in_=_kc_pair_ap(d["xtf8"], Q, hh * 1024, 1024))
        nc.scalar.dma_start(out=wth_sb[:], in_=_kc_pair_ap(d["wthT"], CI, 0, CI))
        if bth_sb is not None:
            nc.scalar.dma_start(out=bth_sb[:], in_=d["bth"][:])
        if bph_sb is not None:
            nc.scalar.dma_start(out=bph_sb[:], in_=d["bph"][:])
        # xbf front quarter on sync (hwdge, lands right after xt so phi-hh0
        # isn't gated on the slow gpsimd swdge path); second quarter swdge
        nc.sync.dma_start(
            out=xbf_t[0][:], in_=_kc_pair_ap(d["xb"], N, 0, 1024))
        nc.gpsimd.dma_start(
            out=xbf_t[1][:], in_=_kc_pair_ap(d["xb"], N, 1024, 1024))

        def xb_at(st128):
            # (tile, local col0) for xb column st128*128
            col = st128 * 128
            if col < 1024:
                return xbf_t[0], col
            if col < 2048:
                return xbf_t[1], col - 1024
            return xbb_sb, col - 2048

        # warm the exp table set early so the first real exp isn't +2.7us
        nc.scalar.activation(scratch[:], scratch[:], AF.Exp, scale=1.0)


        def cast_out(dst_ap, src_psum, bias_part, bias_row):
            # PSUM f32 -> SBUF bf16, optionally + bias
            if bias_part is not None:
                nc.vector.tensor_scalar_add(dst_ap, src_psum, bias_part[:])
            elif bias_row is not None:
                bcast = bass.AP(
                    tensor=bias_row.tensor,
                    offset=bias_row.offset,
                    ap=[[0, 128], [0, 4], [1, CI]],
                )
                nc.vector.tensor_tensor(dst_ap, src_psum, bcast, ALU.add)
            else:
                nc.vector.tensor_copy(dst_ap, src_psum)

        # ---- theta + first quarter of phi (enough for 8 s-tiles) ----
        with (
            tc.tile_pool(name="pj", bufs=2, space="PSUM") as pj,
            tc.tile_pool(name="wup", bufs=1, space="PSUM") as wup,
        ):
            # PE warm-up during the DMA fill: dummy matmuls flip the HAM
            # clock gate toward 8/8 before the first real matmul issues.
            # 8 of them (~4us at the cold clock) end right as the first
            # xt/wth transfers land -- more would push theta out.
            wps = wup.tile([128, 512], F32, tag="wps")
            for _ in range(8):
                nc.tensor.matmul(
                    wps[:, 0:256], ones_sb[:], wup_rhs[:, 0:256],
                    start=True, stop=True)

            def theta_round(hh):
                tp = pj.tile([128, 1024], F32, tag="pj", name=f"tp{hh}")
                for qc in range(2):
                    nc.tensor.matmul(
                        tp[:, ts(qc, 512)],
                        wth_sb[:],
                        xt8_t[hh][:, :, ts(qc, 512)],
                        start=True, stop=True, perf_mode=DR,
                    )
                cast_out(th_sb[:, ts(hh, 1024)], tp[:], bth_sb, None)

            # both theta rounds first (xt lands before the gpsimd xbf
            # transfer), then phi-hh0
            theta_round(0)
            # lazy DMA issues: wph for the phi rounds, wg for the g phase,
            # xb back half for phi hh2/3 + g tiles 16..31
            nc.scalar.dma_start(
                out=wph_sb[:], in_=_kc_pair_ap(d["wphT"], CI, 0, CI))
            nc.scalar.dma_start(
                out=wg_sb[:], in_=_kc_pair_ap(d["wgT"], CI, 0, CI))
            if bg_sb is not None:
                nc.scalar.dma_start(out=bg_sb[:], in_=d["bg"][:])
            nc.sync.dma_start(
                out=xbb_sb[:], in_=_kc_pair_ap(d["xb"], N, 2048, 2048))
            theta_round(1)
            pp = pj.tile([128, 1024], F32, tag="pj")
            for qc in range(2):
                nc.tensor.matmul(
                    pp[:, ts(qc, 512)],
                    wph_sb[:],
                    xbf_t[0][:, :, ts(qc, 512)],
                    start=True, stop=True, perf_mode=DR,
                )
            cast_out(ph_sb[:, 0:1024], pp[:], bph_sb, None)

        # ---- attention, software-pipelined against the remaining
        # projections: exp for s-tile i+8 is emitted behind the y-matmuls
        # of tile i, and the first 8 score/exp pairs precede the phi tail and
        # the whole g^T phase. phi-tail and g^T borrow the yps PSUM banks
        # (the y accumulation's start=True clears them afterwards).
        fF = [
            acts.tile([128, Q], BF16, tag=f"F{j}", name=f"F{j}")
            for j in range(4)
        ]
        # yps as four independent [128,512] chunk tiles: the y main loop's
        # chunk-qc matmul then only WAR-waits the g casts that borrowed
        # THAT chunk (a single [128,2048] tile made y(0) wait all eight)
        ypsp_cm = tc.tile_pool(name="yps", bufs=1, space="PSUM")
        ypsp = ypsp_cm.__enter__()
        yps_t = [
            ypsp.tile([128, 512], F32, tag=f"yps{c}", name=f"yps{c}")
            for c in range(4)
        ]
        scp_cm = tc.tile_pool(name="scp", bufs=2, space="PSUM")
        scp = scp_cm.__enter__()
        bp_cm = tc.tile_pool(name="bp", bufs=1)
        bp = bp_cm.__enter__()
        Bt = {}

        def sc_exp(i):
            B = bp.tile([128, Q], BF16, tag=f"B{i % N_BSLOT}", name=f"B{i}")
            Bt[i] = B
            for h in range(2):
                sc = scp.tile([128, 1024], F32, tag="sc")
                for qc in range(2):
                    nc.tensor.matmul(
                        sc[:, ts(qc, 512)],
                        ph_sb[:, ts(i, 128)],
                        th_sb[:, ts(h * 2 + qc, 512)],
                        start=True,
                        stop=True,
                    )
                if _dve_exp_half(i, h):
                    # Schraudolph exp on DVE: bf16 bits of exp(SCALE*sc)
                    nc.vector.tensor_scalar(
                        B[:, ts(h, 1024)].bitcast(I16), sc[:],
                        SCH_A, SCH_B, ALU.mult, ALU.add)
                else:
                    nc.scalar.activation(
                        B[:, ts(h, 1024)], sc[:], AF.Exp, scale=SCALE)
            # stride-4 fold chains, engine per FOLD_POOL LUT. The first add
            # of a chain sums B[j] and B[j+4] directly -- no seed copies.
            # Tile 31 is NOT folded into chain 3: it becomes the single
            # final add after the chain combines, so the tail critical path
            # past the last exp is one add. DVE folds for tiles 4..7 are
            # emitted later (after the g casts) so the g casts -- which gate
            # the whole y main loop -- aren't queued behind them.
            j = i % 4
            if i < 4 or i == 31:
                pass
            elif i in FOLD_POOL:
                if i == 4:
                    nc.gpsimd.tensor_tensor(fF[j][:], Bt[0][:], Bt[4][:], ALU.add)
                else:
                    nc.gpsimd.tensor_tensor(fF[j][:], fF[j][:], Bt[i][:], ALU.add)
            elif i >= 8:
                nc.vector.tensor_tensor(fF[j][:], fF[j][:], Bt[i][:], ALU.add)

        for i in range(8):
            sc_exp(i)

        # phi tail (tiles 8..31) into borrowed yps chunk tiles
        for hh in range(1, 4):
            c0 = (hh % 2) * 2
            if hh == 1:
                xbt, lc = xbf_t[1], 0
            else:
                xbt, lc = xbb_sb, (hh - 2) * 1024
            for qc in range(2):
                nc.tensor.matmul(
                    yps_t[c0 + qc][:],
                    wph_sb[:],
                    xbt[:, :, lc + qc * 512:lc + qc * 512 + 512],
                    start=True, stop=True, perf_mode=DR,
                )
            for qc in range(2):
                cast_out(
                    ph_sb[:, hh * 1024 + qc * 512:hh * 1024 + qc * 512 + 512],
                    yps_t[c0 + qc][:], bph_sb, None)

        # g^T projection, also into borrowed yps banks. The casts split
        # between DVE and ACT: the y main loop can only begin once ALL g
        # casts have drained the borrowed banks, and DVE alone finishes
        # ~2us after the g matmuls do.
        for b in range(8):
            gp = yps_t[b % 4][:]
            for sj in range(4):
                st = b * 4 + sj
                xbt, lc = xb_at(st)
                nc.tensor.matmul(
                    gp[:, ts(sj, 128)],
                    xbt[:, :, lc:lc + 128],
                    wg_sb[:],
                    start=True, stop=True, perf_mode=DR,
                )
            if bg_sb is None and b % 2 == 1:
                nc.scalar.copy(gT_sb[:, ts(b, 512)], gp[:])
            else:
                cast_out(gT_sb[:, ts(b, 512)], gp[:], None, bg_sb)

        # deferred DVE first-adds for chains 1..3 (tiles 5,6,7), after the
        # g casts so those never wait behind exp-gated folds
        for i in (5, 6, 7):
            nc.vector.tensor_tensor(fF[i % 4][:], Bt[i % 4][:], Bt[i][:], ALU.add)

        # ---- main loop: y(i) first, then scores/exp for i+8, so the PE
        # queue head never blocks on an exp that is still in flight ----
        for i in range(32):
            B = Bt[i]
            for h in range(2):
                for qc in range(2):
                    nc.tensor.matmul(
                        yps_t[h * 2 + qc][:],
                        gT_sb[:, ts(i, 128)],
                        B[:, ts(h * 2 + qc, 512)],
                        start=(i == 0),
                        stop=(i == 31),
                    )
            if i == 0:
                # tail-only weights, issued now so they never gate the
                # projections' DMA waits but still land long before use
                nc.sync.dma_start(out=wo_sb[:], in_=d["woT"][:])
                nc.sync.dma_start(out=id_sb[:], in_=d["ident"][:])
                for hh in range(2):
                    nc.sync.dma_start(
                        out=xt_t[hh][:],
                        in_=_kc_pair_ap(d["xt"], Q, hh * 1024, 1024))
                if bo_sb is not None:
                    for oc in range(2):
                        nc.sync.dma_start(out=bo_sb[:, oc, :], in_=d["bo"][oc])
            if i < 24:
                sc_exp(i + 8)
        scp_cm.__exit__(None, None, None)

        # ---- tail, pipelined per 512-wide q-chunk:
        # d = ones @ {F0..F3, B31} (5-deep PSUM accumulation -- no DVE
        # combine chain at all) -> 1/d (approx) -> y*1/d -> {residual +
        # out-proj as one PSUM accumulation: ident@xt (start) + woT@ynt
        # (stop)} -> copy to SBUF -> DMA out.
        with (
            tc.tile_pool(name="dps", bufs=2, space="PSUM") as dpsp,
            tc.tile_pool(name="rps", bufs=2, space="PSUM") as rps,
        ):
            # dep-free dummy matmuls keep the PE busy while the last
            # exps/folds drain, so the HAM clock stays at 8/8 for the
            # tail's small matmuls (measured 2x difference)
            for _ in range(6):
                dwm = dpsp.tile([128, 512], F32, tag="dp")
                nc.tensor.matmul(
                    dwm[:, 0:256], ones_sb[:], wup_rhs[:, 0:256],
                    start=True, stop=True)
            for qc in range(4):
                dp = dpsp.tile([128, 512], F32, tag="dp")
                for fi in range(5):
                    src = fF[fi][:, ts(qc, 512)] if fi < 4 else (
                        Bt[31][:, ts(qc, 512)])
                    nc.tensor.matmul(
                        dp[:], ones_sb[:], src,
                        start=(fi == 0), stop=(fi == 4))
                rcp = outp.tile([128, 512], F32, tag="rcp")
                nc.vector.reciprocal_approx_fast(rcp[:], dp[:])
                ynt = outp.tile([128, 512], BF16, tag="ynt")
                nc.vector.tensor_tensor(
                    ynt[:], yps_t[qc][:], rcp[:], ALU.mult)
                for oc in range(2):
                    rp = rps.tile([128, 512], F32, tag="rp")
                    xres = xt_t[qc // 2][:, oc, ts(qc % 2, 512)]
                    nc.tensor.matmul(
                        rp[:], id_sb[:], xres,
                        start=True, stop=False)
                    nc.tensor.matmul(
                        rp[:],
                        wo_sb[:, ts(oc, 128)],
                        ynt[:],
                        start=False,
                        stop=True,
                    )
                    # PSUM -> SBUF split across ACT (oc0) and DVE (oc1) so
                    # the copies run concurrently (DMA cannot read PSUM);
                    # out DMAs split across the sync and scalar queues
                    ot = outp.tile([128, 512], F32, tag=f"ot{oc}")
                    if bo_sb is not None:
                        nc.scalar.activation(
                            ot[:], rp[:], AF.Identity, bias=bo_sb[:, oc, :])
                    elif oc == 0:
                        nc.scalar.copy(ot[:], rp[:])
                    else:
                        nc.vector.tensor_copy(ot[:], rp[:])
                    [nc.sync, nc.scalar][oc].dma_start(
                        out=d["out"][oc][:, ts(qc, 512)], in_=ot[:])
        bp_cm.__exit__(None, None, None)
        ypsp_cm.__exit__(None, None, None)
        xin_cm.__exit__(None, None, None)


def _prep_in_maps(inputs):
    bf = ml_dtypes.bfloat16
    x = np.ascontiguousarray(np.asarray(inputs["x"], dtype=np.float32))
    w_g = np.asarray(inputs["w_g"], np.float32)
    b_g = np.asarray(inputs["b_g"], np.float32)
    w_theta = np.asarray(inputs["w_theta"], np.float32)
    b_theta = np.asarray(inputs["b_theta"], np.float32)
    w_phi = np.asarray(inputs["w_phi"], np.float32)
    b_phi = np.asarray(inputs["b_phi"], np.float32)
    w_out = np.asarray(inputs["w_out"], np.float32)
    b_out = np.asarray(inputs["b_out"], np.float32)

    flags = (
        bool(np.any(b_theta)), bool(np.any(b_phi)),
        bool(np.any(b_g)), bool(np.any(b_out)),
    )
    f8 = ml_dtypes.float8_e4m3
    wthT = np.ascontiguousarray(w_theta.T).astype(f8).reshape(2, 128, CI)
    wphT = np.ascontiguousarray(w_phi.T).astype(f8).reshape(2, 128, CI)
    wgT = np.ascontiguousarray(w_g.T).astype(f8).reshape(2, 128, CI)
    woT = np.ascontiguousarray(w_out.T).astype(bf)          # [CI, C]
    ident = np.eye(128, dtype=bf)

    in_maps = []
    for c in range(NCORES):
        n, qh = c // 2, c % 2
        xr = x[n].reshape(C, N)
        xbc = xr.astype(bf)
        x8 = xr.astype(f8)
        m = {
            "xb": np.ascontiguousarray(x8.reshape(2, 128, N)),
            "xt": np.ascontiguousarray(
                xbc[:, qh * Q:(qh + 1) * Q].reshape(2, 128, Q)),
            "xtf8": np.ascontiguousarray(
                x8[:, qh * Q:(qh + 1) * Q].reshape(2, 128, Q)),
            "wthT": wthT, "wphT": wphT, "wgT": wgT, "woT": woT,
            "ident": ident,
        }
        if flags[0]:
            m["bth"] = np.ascontiguousarray(b_theta.reshape(128, 1))
        if flags[1]:
            m["bph"] = np.ascontiguousarray(b_phi.reshape(128, 1))
        if flags[2]:
            m["bg"] = np.ascontiguousarray(b_g.reshape(1, CI))
        if flags[3]:
            m["bo"] = np.ascontiguousarray(b_out.reshape(2, 128, 1))
        in_maps.append(m)
    return flags, in_maps


def _get_nc(flags):
    if flags not in _CACHE:
        _CACHE[flags] = _build(flags)
    return _CACHE[flags]


def kernel(**inputs):
    flags, in_maps = _prep_in_maps(inputs)
    nc = _get_nc(flags)
    res = run_bass_kernel_spmd(nc, in_maps, list(range(NCORES)))
    out = np.empty((NB, C, N), np.float32)
    for c in range(NCORES):
        n, qh = c // 2, c % 2
        out[n][:, qh * Q:(qh + 1) * Q] = res.results[c]["out"].reshape(C, Q)
    return out.reshape(NB, C, 64, 64)


if __name__ == "__main__":
    x = np.random.randn(NB, C, 64, 64).astype(np.float32) * 0.1
    rng = np.random.default_rng(0)
    ins = {
        "x": x,
        "w_g": rng.normal(size=(CI, C)).astype(np.float32) * 0.01,
        "b_g": np.zeros(CI, np.float32),
        "w_theta": rng.normal(size=(CI, C)).astype(np.float32) * 0.01,
        "b_theta": np.zeros(CI, np.float32),
        "w_phi": rng.normal(size=(CI, C)).astype(np.float32) * 0.01,
        "b_phi": np.zeros(CI, np.float32),
        "w_out": rng.normal(size=(C, CI)).astype(np.float32) * 0.01,
        "b_out": np.zeros(C, np.float32),
    }
    o = kernel(**ins)
    print("ok", o.shape, o.dtype)


# revision 45
# speedup vs baseline: 1.3175x; 1.0083x over previous
"""NonLocal2D (attention) block on 8 trn2 NeuronCores.

Sharding: core c -> batch n = c//2, query-half qh = c%2 (2048 of the 4096
spatial positions). Each core receives the full x[n] (so phi/g are computed
locally -- no collectives) plus its own query slice, and produces
out[n][:, qh*2048:(qh+1)*2048].

Per-core dataflow (layouts chosen so no transposes are ever needed):
  theta:    [CI=128, Q]  = wthT-chunks (lhsT) @ xt-chunks (rhs)     [PE]
  phi:      [CI=128, N]  = wphT-chunks (lhsT) @ xb-chunks (rhs)     [PE]
  g^T:      [s, CI] tiles = xb-chunks (lhsT) @ wgT-chunks (rhs)     [PE]
  scores^T: [s=128, q=1024] = phi-tile (lhsT) @ theta (rhs)         [PE -> PSUM f32]
  B = exp(SCALE*scores^T) -> bf16 SBUF; most half-tiles on ACT, a
      subset on DVE via the Schraudolph int16-bits trick (bf16 bits =
      trunc(x*128/ln2 + 127*128 - c))  (no max-sub: |scaled| < ~30)  [ACT+DVE]
  denom: 4 stride-4 fold chains over B tiles (split GPSIMD/DVE via a
      static LUT; the first add of each chain combines two B tiles so
      no copies are needed), combined to one F, then
      d = ones (lhsT) @ F per q-chunk                               [DVE/GPSIMD/PE]
  y^T += gT-tile (lhsT) @ B   (PSUM accumulate over 32 s-tiles)     [PE]
  y_norm^T = y^T * (1/d) -> bf16                                    [DVE]
  out-proj + residual in one PSUM group:
      rp = ident (lhsT) @ xt-chunk  (start)  -- the +x residual
      rp += woT-chunk (lhsT) @ y_norm^T (stop); DMA out from PSUM   [PE]

The residual uses the bf16 xt copy (no separate f32 x load): the extra
~2^-9 relative rounding on x costs ~1.7e-3 rel err, far under the 2e-2
gate, and saves 4MB/core of input DMA.
"""

import numpy as np
import ml_dtypes

import concourse.bass as bass
import concourse.mybir as mybir
import concourse.tile as tile
from concourse import bacc
from concourse.bass import ts
from concourse.bass_utils import run_bass_kernel_spmd

BF16 = mybir.dt.bfloat16
F32 = mybir.dt.float32
F8 = mybir.dt.float8e4
I16 = mybir.dt.int16
DR = mybir.MatmulPerfMode.DoubleRow
AF = mybir.ActivationFunctionType
ALU = mybir.AluOpType

C = 256          # in channels
CI = 128         # inter channels
NB = 4           # batch
N = 4096         # H*W
Q = 2048         # queries per core
NCORES = 8
SCALE = float(128 ** 0.5)   # reference divides by d**-0.5

# Schraudolph exp in bf16-bit space: bits = trunc(x*SCALE*128/ln2 + b)
SCH_A = SCALE * 128.0 / float(np.log(2.0))
SCH_B = 127.0 * 128.0 - 5.0

N_BSLOT = 24     # B-tile ring (WAR distance 24 >> pipeline depth)

# fold engine LUT: which tiles' fold-adds run on GPSIMD (rest on DVE).
# Exactly chain 0 (j = i%4 == 0): a whole chain per engine, so no
# cross-engine handoff ever head-blocks the DVE queue.
FOLD_POOL = {4, 8, 12, 16, 20, 24, 28}

_CACHE: dict = {}


def _dve_exp_half(i, h):
    # which exp half-tiles run on DVE (Schraudolph) instead of ACT.
    # Tiles 30/31 stay fully on ACT (the last exps gate the tail and DVE
    # still owes their folds); both halves of 29 go to DVE so ACT
    # reaches the final exps sooner.
    return (h == 0 and i % 3 == 1 and i < 29) or i == 29 or (i == 30 and h == 0)


def _build(flags):
    bth_nz, bph_nz, bg_nz, bo_nz = flags
    nc = bacc.Bacc("TRN2", target_bir_lowering=False, debug=False)

    d = {}
    d["xb"] = nc.dram_tensor("xb", [2, 128, N], F8, kind="ExternalInput").ap()
    d["xt"] = nc.dram_tensor("xt", [2, 128, Q], BF16, kind="ExternalInput").ap()
    d["xtf8"] = nc.dram_tensor("xtf8", [2, 128, Q], F8, kind="ExternalInput").ap()
    d["wthT"] = nc.dram_tensor("wthT", [2, 128, CI], F8, kind="ExternalInput").ap()
    d["wphT"] = nc.dram_tensor("wphT", [2, 128, CI], F8, kind="ExternalInput").ap()
    d["wgT"] = nc.dram_tensor("wgT", [2, 128, CI], F8, kind="ExternalInput").ap()
    d["woT"] = nc.dram_tensor("woT", [128, C], BF16, kind="ExternalInput").ap()
    d["ident"] = nc.dram_tensor("ident", [128, 128], BF16, kind="ExternalInput").ap()
    d["bth"] = nc.dram_tensor("bth", [128, 1], F32, kind="ExternalInput").ap() if bth_nz else None
    d["bph"] = nc.dram_tensor("bph", [128, 1], F32, kind="ExternalInput").ap() if bph_nz else None
    d["bg"] = nc.dram_tensor("bg", [1, CI], F32, kind="ExternalInput").ap() if bg_nz else None
    d["bo"] = nc.dram_tensor("bo", [2, 128, 1], F32, kind="ExternalInput").ap() if bo_nz else None
    d["out"] = nc.dram_tensor("out", [2, 128, Q], F32, kind="ExternalOutput").ap()

    with tile.TileContext(nc) as tc:
        _bass_body(tc, d)
    nc.compile()
    return nc


def _kc_pair_ap(dram_ap, cols, col0, count):
    """3D dram AP reading [2,128,cols] as [p=128, kc=2, count] at col0."""
    return bass.AP(
        tensor=dram_ap.tensor,
        offset=col0,
        ap=[[cols, 128], [128 * cols, 2], [1, count]],
    )


def _bass_body(tc, d):
    nc = tc.nc

    with (
        tc.tile_pool(name="const", bufs=1) as const,
        tc.tile_pool(name="acts", bufs=1) as acts,
        tc.tile_pool(name="outs", bufs=2) as outp,
    ):
        # ---- constants / weights ----
        # memsets on DVE (idle at t=0) so the PE warm-up never waits on the
        # gpsimd program load; the exp-table warm reads scratch itself
        # (garbage in, table warmed) so it carries no cross-engine dep
        ones_sb = const.tile([128, 128], BF16, tag="ones")
        nc.vector.memset(ones_sb[:], 1.0)
        wup_rhs = const.tile([128, 512], BF16, tag="wup_rhs")
        nc.vector.memset(wup_rhs[:], 0.0)
        scratch = const.tile([128, 1], BF16, tag="scratch")
        nc.vector.memset(scratch[:], 1.0)

        wth_sb = const.tile([128, 2, CI], F8, tag="wth")
        wph_sb = const.tile([128, 2, CI], F8, tag="wph")
        wg_sb = const.tile([128, 2, CI], F8, tag="wg")
        wo_sb = const.tile([128, C], BF16, tag="wo")
        id_sb = const.tile([128, 128], BF16, tag="ident")
        bth_sb = bph_sb = bg_sb = bo_sb = None
        if d["bth"] is not None:
            bth_sb = const.tile([128, 1], F32, tag="bth")
        if d["bph"] is not None:
            bph_sb = const.tile([128, 1], F32, tag="bph")
        if d["bg"] is not None:
            bg_sb = const.tile([1, CI], F32, tag="bg")
        if d["bo"] is not None:
            bo_sb = const.tile([128, 2, 1], F32, tag="bo")

        th_sb = acts.tile([128, Q], BF16, tag="th")
        ph_sb = acts.tile([128, N], BF16, tag="ph")
        gT_sb = acts.tile([128, 32 * CI], BF16, tag="gT")  # tile i at cols [128i, 128i+128)

        # ---- input fill: one tile per DMA transfer so every consumer's
        # RAW dep is exactly one transfer (no waiting on sibling chunks),
        # ordered by first use across three queues ----
        xin_cm = tc.tile_pool(name="xin", bufs=1)
        xin = xin_cm.__enter__()
        xt_t = [
            xin.tile([128, 2, 1024], BF16, tag=f"xt{hh}", name=f"xt{hh}")
            for hh in range(2)
        ]
        xt8_t = [
            xin.tile([128, 2, 1024], F8, tag=f"xt8{hh}", name=f"xt8{hh}")
            for hh in range(2)
        ]
        xbf_t = [
            xin.tile([128, 2, 1024], F8, tag=f"xbf{c}", name=f"xbf{c}")
            for c in range(2)
        ]
        xbb_sb = xin.tile([128, 2, 2048], F8, tag="xbb")

        # Only the transfers needed FIRST are issued here. DMA waits appear
        # to coalesce to "every transfer issued so far on that queue", so
        # later-needed tensors (wg, wo, ident, xb back half, biases) are
        # issued lazily at their point of need to keep theta's wait minimal.
        # sync: xt halves (theta); scalar: wth; gpsimd: front half of xb
        for hh in range(2):
            nc.sync.dma_start(
                out=xt8_t[hh][:],
                in_=_kc_pair_ap(d["xtf8"], Q, hh * 1024, 1024))
        nc.scalar.dma_start(out=wth_sb[:], in_=_kc_pair_ap(d["wthT"], CI, 0, CI))
        if bth_sb is not None:
            nc.scalar.dma_start(out=bth_sb[:], in_=d["bth"][:])
        if bph_sb is not None:
            nc.scalar.dma_start(out=bph_sb[:], in_=d["bph"][:])
        # xbf front quarter on sync (hwdge, lands right after xt so phi-hh0
        # isn't gated on the slow gpsimd swdge path); second quarter swdge
        nc.sync.dma_start(
            out=xbf_t[0][:], in_=_kc_pair_ap(d["xb"], N, 0, 1024))
        nc.gpsimd.dma_start(
            out=xbf_t[1][:], in_=_kc_pair_ap(d["xb"], N, 1024, 1024))

        def xb_at(st128):
            # (tile, local col0) for xb column st128*128
            col = st128 * 128
            if col < 1024:
                return xbf_t[0], col
            if col < 2048:
                return xbf_t[1], col - 1024
            return xbb_sb, col - 2048

        # warm the exp table set early so the first real exp isn't +2.7us
        nc.scalar.activation(scratch[:], scratch[:], AF.Exp, scale=1.0)


        def cast_out(dst_ap, src_psum, bias_part, bias_row):
            # PSUM f32 -> SBUF bf16, optionally + bias
            if bias_part is not None:
                nc.vector.tensor_scalar_add(dst_ap, src_psum, bias_part[:])
            elif bias_row is not None:
                bcast = bass.AP(
                    tensor=bias_row.tensor,
                    offset=bias_row.offset,
                    ap=[[0, 128], [0, 4], [1, CI]],
                )
                nc.vector.tensor_tensor(dst_ap, src_psum, bcast, ALU.add)
            else:
                nc.vector.tensor_copy(dst_ap, src_psum)

        # ---- theta + first quarter of phi (enough for 8 s-tiles) ----
        with (
            tc.tile_pool(name="pj", bufs=2, space="PSUM") as pj,
            tc.tile_pool(name="wup", bufs=1, space="PSUM") as wup,
        ):
            # PE warm-up during the DMA fill: dummy matmuls flip the HAM
            # clock gate toward 8/8 before the first real matmul issues.
            # 8 of them (~4us at the cold clock) end right as the first
            # xt/wth transfers land -- more would push theta out.
            wps = wup.tile([128, 512], F32, tag="wps")
            for _ in range(10):
                nc.tensor.matmul(
                    wps[:, 0:256], ones_sb[:], wup_rhs[:, 0:256],
                    start=True, stop=True)

            def theta_round(hh):
                tp = pj.tile([128, 1024], F32, tag="pj", name=f"tp{hh}")
                for qc in range(2):
                    nc.tensor.matmul(
                        tp[:, ts(qc, 512)],
                        wth_sb[:],
                        xt8_t[hh][:, :, ts(qc, 512)],
                        start=True, stop=True, perf_mode=DR,
                    )
                cast_out(th_sb[:, ts(hh, 1024)], tp[:], bth_sb, None)

            # both theta rounds first (xt lands before the gpsimd xbf
            # transfer), then phi-hh0
            theta_round(0)
            # lazy DMA issues: wph for the phi rounds, wg for the g phase,
            # xb back half for phi hh2/3 + g tiles 16..31
            nc.scalar.dma_start(
                out=wph_sb[:], in_=_kc_pair_ap(d["wphT"], CI, 0, CI))
            nc.scalar.dma_start(
                out=wg_sb[:], in_=_kc_pair_ap(d["wgT"], CI, 0, CI))
            if bg_sb is not None:
                nc.scalar.dma_start(out=bg_sb[:], in_=d["bg"][:])
            nc.sync.dma_start(
                out=xbb_sb[:], in_=_kc_pair_ap(d["xb"], N, 2048, 2048))
            theta_round(1)
            pp = pj.tile([128, 1024], F32, tag="pj")
            for qc in range(2):
                nc.tensor.matmul(
                    pp[:, ts(qc, 512)],
                    wph_sb[:],
                    xbf_t[0][:, :, ts(qc, 512)],
                    start=True, stop=True, perf_mode=DR,
                )
            cast_out(ph_sb[:, 0:1024], pp[:], bph_sb, None)

        # ---- attention, software-pipelined against the remaining
        # projections: exp for s-tile i+8 is emitted behind the y-matmuls
        # of tile i, and the first 8 score/exp pairs precede the phi tail and
        # the whole g^T phase. phi-tail and g^T borrow the yps PSUM banks
        # (the y accumulation's start=True clears them afterwards).
        fF = [
            acts.tile([128, Q], BF16, tag=f"F{j}", name=f"F{j}")
            for j in range(4)
        ]
        # yps as four independent [128,512] chunk tiles: the y main loop's
        # chunk-qc matmul then only WAR-waits the g casts that borrowed
        # THAT chunk (a single [128,2048] tile made y(0) wait all eight)
        ypsp_cm = tc.tile_pool(name="yps", bufs=1, space="PSUM")
        ypsp = ypsp_cm.__enter__()
        yps_t = [
            ypsp.tile([128, 512], F32, tag=f"yps{c}", name=f"yps{c}")
            for c in range(4)
        ]
        scp_cm = tc.tile_pool(name="scp", bufs=2, space="PSUM")
        scp = scp_cm.__enter__()
        bp_cm = tc.tile_pool(name="bp", bufs=1)
        bp = bp_cm.__enter__()
        Bt = {}

        def sc_exp(i):
            B = bp.tile([128, Q], BF16, tag=f"B{i % N_BSLOT}", name=f"B{i}")
            Bt[i] = B
            for h in range(2):
                sc = scp.tile([128, 1024], F32, tag="sc")
                for qc in range(2):
                    nc.tensor.matmul(
                        sc[:, ts(qc, 512)],
                        ph_sb[:, ts(i, 128)],
                        th_sb[:, ts(h * 2 + qc, 512)],
                        start=True,
                        stop=True,
                    )
                if _dve_exp_half(i, h):
                    # Schraudolph exp on DVE: bf16 bits of exp(SCALE*sc)
                    nc.vector.tensor_scalar(
                        B[:, ts(h, 1024)].bitcast(I16), sc[:],
                        SCH_A, SCH_B, ALU.mult, ALU.add)
                else:
                    nc.scalar.activation(
                        B[:, ts(h, 1024)], sc[:], AF.Exp, scale=SCALE)
            # stride-4 fold chains, engine per FOLD_POOL LUT. The first add
            # of a chain sums B[j] and B[j+4] directly -- no seed copies.
            # Tile 31 is NOT folded into chain 3: it becomes the single
            # final add after the chain combines, so the tail critical path
            # past the last exp is one add. DVE folds for tiles 4..7 are
            # emitted later (after the g casts) so the g casts -- which gate
            # the whole y main loop -- aren't queued behind them.
            j = i % 4
            if i < 4 or i == 31:
                pass
            elif i in FOLD_POOL:
                if i == 4:
                    nc.gpsimd.tensor_tensor(fF[j][:], Bt[0][:], Bt[4][:], ALU.add)
                else:
                    nc.gpsimd.tensor_tensor(fF[j][:], fF[j][:], Bt[i][:], ALU.add)
            elif i >= 8:
                nc.vector.tensor_tensor(fF[j][:], fF[j][:], Bt[i][:], ALU.add)

        for i in range(8):
            sc_exp(i)

        # phi tail (tiles 8..31) into borrowed yps chunk tiles
        for hh in range(1, 4):
            c0 = (hh % 2) * 2
            if hh == 1:
                xbt, lc = xbf_t[1], 0
            else:
                xbt, lc = xbb_sb, (hh - 2) * 1024
            for qc in range(2):
                nc.tensor.matmul(
                    yps_t[c0 + qc][:],
                    wph_sb[:],
                    xbt[:, :, lc + qc * 512:lc + qc * 512 + 512],
                    start=True, stop=True, perf_mode=DR,
                )
            for qc in range(2):
                cast_out(
                    ph_sb[:, hh * 1024 + qc * 512:hh * 1024 + qc * 512 + 512],
                    yps_t[c0 + qc][:], bph_sb, None)

        # g^T projection, also into borrowed yps banks. The casts split
        # between DVE and ACT: the y main loop can only begin once ALL g
        # casts have drained the borrowed banks, and DVE alone finishes
        # ~2us after the g matmuls do.
        for b in range(8):
            gp = yps_t[b % 4][:]
            for sj in range(4):
                st = b * 4 + sj
                xbt, lc = xb_at(st)
                nc.tensor.matmul(
                    gp[:, ts(sj, 128)],
                    xbt[:, :, lc:lc + 128],
                    wg_sb[:],
                    start=True, stop=True, perf_mode=DR,
                )
            if bg_sb is None and b % 2 == 1:
                nc.scalar.copy(gT_sb[:, ts(b, 512)], gp[:])
            else:
                cast_out(gT_sb[:, ts(b, 512)], gp[:], None, bg_sb)

        # deferred DVE first-adds for chains 1..3 (tiles 5,6,7), after the
        # g casts so those never wait behind exp-gated folds
        for i in (5, 6, 7):
            nc.vector.tensor_tensor(fF[i % 4][:], Bt[i % 4][:], Bt[i][:], ALU.add)

        # ---- main loop: y(i) first, then scores/exp for i+8, so the PE
        # queue head never blocks on an exp that is still in flight ----
        for i in range(32):
            B = Bt[i]
            for h in range(2):
                for qc in range(2):
                    nc.tensor.matmul(
                        yps_t[h * 2 + qc][:],
                        gT_sb[:, ts(i, 128)],
                        B[:, ts(h * 2 + qc, 512)],
                        start=(i == 0),
                        stop=(i == 31),
                    )
            if i == 0:
                # tail-only weights, issued now so they never gate the
                # projections' DMA waits but still land long before use
                nc.sync.dma_start(out=wo_sb[:], in_=d["woT"][:])
                nc.sync.dma_start(out=id_sb[:], in_=d["ident"][:])
                for hh in range(2):
                    nc.sync.dma_start(
                        out=xt_t[hh][:],
                        in_=_kc_pair_ap(d["xt"], Q, hh * 1024, 1024))
                if bo_sb is not None:
                    for oc in range(2):
                        nc.sync.dma_start(out=bo_sb[:, oc, :], in_=d["bo"][oc])
            if i < 24:
                sc_exp(i + 8)
        scp_cm.__exit__(None, None, None)

        # ---- tail, pipelined per 512-wide q-chunk:
        # d = ones @ {F0..F3, B31} (5-deep PSUM accumulation -- no DVE
        # combine chain at all) -> 1/d (approx) -> y*1/d -> {residual +
        # out-proj as one PSUM accumulation: ident@xt (start) + woT@ynt
        # (stop)} -> copy to SBUF -> DMA out.
        with (
            tc.tile_pool(name="dps", bufs=2, space="PSUM") as dpsp,
            tc.tile_pool(name="rps", bufs=2, space="PSUM") as rps,
        ):
            # dep-free dummy matmuls keep the PE busy while the last
            # exps/folds drain, so the HAM clock stays at 8/8 for the
            # tail's small matmuls (measured 2x difference)
            for _ in range(6):
                dwm = dpsp.tile([128, 512], F32, tag="dp")
                nc.tensor.matmul(
                    dwm[:, 0:256], ones_sb[:], wup_rhs[:, 0:256],
                    start=True, stop=True)
            for qc in range(4):
                dp = dpsp.tile([128, 512], F32, tag="dp")
                for fi in range(5):
                    src = fF[fi][:, ts(qc, 512)] if fi < 4 else (
                        Bt[31][:, ts(qc, 512)])
                    nc.tensor.matmul(
                        dp[:], ones_sb[:], src,
                        start=(fi == 0), stop=(fi == 4))
                rcp = outp.tile([128, 512], F32, tag="rcp")
                nc.vector.reciprocal_approx_fast(rcp[:], dp[:])
                ynt = outp.tile([128, 512], BF16, tag="ynt")
                nc.vector.tensor_tensor(
                    ynt[:], yps_t[qc][:], rcp[:], ALU.mult)
                for oc in range(2):
                    rp = rps.tile([128, 512], F32, tag="rp")
                    xres = xt_t[qc // 2][:, oc, ts(qc % 2, 512)]
                    nc.tensor.matmul(
                        rp[:], id_sb[:], xres,
                        start=True, stop=False)
                    nc.tensor.matmul(
                        rp[:],
                        wo_sb[:, ts(oc, 128)],
                        ynt[:],
                        start=False,
                        stop=True,
                    )
                    # PSUM -> SBUF split across ACT (oc0) and DVE (oc1) so
                    # the copies run concurrently (DMA cannot read PSUM);
                    # out DMAs split across the sync and scalar queues
                    ot = outp.tile([128, 512], F32, tag=f"ot{oc}")
                    if bo_sb is not None:
                        nc.scalar.activation(
                            ot[:], rp[:], AF.Identity, bias=bo_sb[:, oc, :])
                    elif oc == 0:
                        nc.scalar.copy(ot[:], rp[:])
                    else:
                        nc.vector.tensor_copy(ot[:], rp[:])
                    [nc.sync, nc.scalar][oc].dma_start(
                        out=d["out"][oc][:, ts(qc, 512)], in_=ot[:])
        bp_cm.__exit__(None, None, None)
        ypsp_cm.__exit__(None, None, None)
        xin_cm.__exit__(None, None, None)


def _prep_in_maps(inputs):
    bf = ml_dtypes.bfloat16
    x = np.ascontiguousarray(np.asarray(inputs["x"], dtype=np.float32))
    w_g = np.asarray(inputs["w_g"], np.float32)
    b_g = np.asarray(inputs["b_g"], np.float32)
    w_theta = np.asarray(inputs["w_theta"], np.float32)
    b_theta = np.asarray(inputs["b_theta"], np.float32)
    w_phi = np.asarray(inputs["w_phi"], np.float32)
    b_phi = np.asarray(inputs["b_phi"], np.float32)
    w_out = np.asarray(inputs["w_out"], np.float32)
    b_out = np.asarray(inputs["b_out"], np.float32)

    flags = (
        bool(np.any(b_theta)), bool(np.any(b_phi)),
        bool(np.any(b_g)), bool(np.any(b_out)),
    )
    f8 = ml_dtypes.float8_e4m3
    wthT = np.ascontiguousarray(w_theta.T).astype(f8).reshape(2, 128, CI)
    wphT = np.ascontiguousarray(w_phi.T).astype(f8).reshape(2, 128, CI)
    wgT = np.ascontiguousarray(w_g.T).astype(f8).reshape(2, 128, CI)
    woT = np.ascontiguousarray(w_out.T).astype(bf)          # [CI, C]
    ident = np.eye(128, dtype=bf)

    in_maps = []
    for c in range(NCORES):
        n, qh = c // 2, c % 2
        xr = x[n].reshape(C, N)
        xbc = xr.astype(bf)
        x8 = xr.astype(f8)
        m = {
            "xb": np.ascontiguousarray(x8.reshape(2, 128, N)),
            "xt": np.ascontiguousarray(
                xbc[:, qh * Q:(qh + 1) * Q].reshape(2, 128, Q)),
            "xtf8": np.ascontiguousarray(
                x8[:, qh * Q:(qh + 1) * Q].reshape(2, 128, Q)),
            "wthT": wthT, "wphT": wphT, "wgT": wgT, "woT": woT,
            "ident": ident,
        }
        if flags[0]:
            m["bth"] = np.ascontiguousarray(b_theta.reshape(128, 1))
        if flags[1]:
            m["bph"] = np.ascontiguousarray(b_phi.reshape(128, 1))
        if flags[2]:
            m["bg"] = np.ascontiguousarray(b_g.reshape(1, CI))
        if flags[3]:
            m["bo"] = np.ascontiguousarray(b_out.reshape(2, 128, 1))
        in_maps.append(m)
    return flags, in_maps


def _get_nc(flags):
    if flags not in _CACHE:
        _CACHE[flags] = _build(flags)
    return _CACHE[flags]


def kernel(**inputs):
    flags, in_maps = _prep_in_maps(inputs)
    nc = _get_nc(flags)
    res = run_bass_kernel_spmd(nc, in_maps, list(range(NCORES)))
    out = np.empty((NB, C, N), np.float32)
    for c in range(NCORES):
        n, qh = c // 2, c % 2
        out[n][:, qh * Q:(qh + 1) * Q] = res.results[c]["out"].reshape(C, Q)
    return out.reshape(NB, C, 64, 64)


if __name__ == "__main__":
    x = np.random.randn(NB, C, 64, 64).astype(np.float32) * 0.1
    rng = np.random.default_rng(0)
    ins = {
        "x": x,
        "w_g": rng.normal(size=(CI, C)).astype(np.float32) * 0.01,
        "b_g": np.zeros(CI, np.float32),
        "w_theta": rng.normal(size=(CI, C)).astype(np.float32) * 0.01,
        "b_theta": np.zeros(CI, np.float32),
        "w_phi": rng.normal(size=(CI, C)).astype(np.float32) * 0.01,
        "b_phi": np.zeros(CI, np.float32),
        "w_out": rng.normal(size=(C, CI)).astype(np.float32) * 0.01,
        "b_out": np.zeros(C, np.float32),
    }
    o = kernel(**ins)
    print("ok", o.shape, o.dtype)
